# revision 16
# baseline (speedup 1.0000x reference)
"""AutoCorrelation layer kernel for 8 Trainium2 NeuronCores.

Math note: the reference's rfft/irfft pair over the zero-padded head dim
computes a circular cross-correlation; its mean over all lags collapses
analytically to (sum_d q_proj) * (sum_d k_proj) per head.  So
corr_mean[b,l] = (1/(H*L)) * sum_h (q[b,l] @ WqS + bqS)_h * (k[b,l] @ WkS + bkS)_h
with WqS = Wq.reshape(D,H,DK).sum(-1).  Everything downstream (top-6,
softmax, gather, output projection) follows the reference directly.

Distribution: all 8 cores redundantly compute the cheap preprocessing
(full q/k/v, ~24MB) and each core computes its own column shard of the
huge (256, 262144) output projection (column-parallel, no collectives).
"""
import sys
import types

sys.path.insert(0, "/opt/trn_rl_repo")

import numpy as np
import concourse.bass as bass
import concourse.mybir as mybir
import concourse.tile as tile
from concourse import bacc
from concourse.bass_utils import run_bass_kernel_spmd
from concourse.masks import make_identity

F32 = mybir.dt.float32
BF16 = mybir.dt.bfloat16

N_CORES = 8
B, L, D, H, DK = 8, 1024, 256, 8, 32
K_TOP = 6
NSH = (L * D) // N_CORES          # 32768 output cols per core
TILE_N = 2048
N_TILES = NSH // TILE_N           # 16
SUBS = TILE_N // 512              # 4
SCALE = 1.0 / (H * L)

WP_BUFS = 6
MODE = "rep"  # "dp" = batch-parallel preproc + AllGather; "rep" = redundant preproc
DEBUG_OUTS = False
TRACE = False          # test harness sets this for profiled runs
LAST_RESULT = None     # stashed BassKernelResults from the last kernel() call

_CACHE = {}


def _build_nc():
    nc = bacc.Bacc("TRN2", target_bir_lowering=False, debug=False, num_devices=N_CORES)

    qt_d = nc.dram_tensor("qt", [B * D, L], F32, kind="ExternalInput").ap()
    kt_d = nc.dram_tensor("kt", [B * D, L], F32, kind="ExternalInput").ap()
    v_d = nc.dram_tensor("v", [B * L, D], F32, kind="ExternalInput").ap()
    wq_d = nc.dram_tensor("wq", [D, D], F32, kind="ExternalInput").ap()
    wk_d = nc.dram_tensor("wk", [D, D], F32, kind="ExternalInput").ap()
    wv_d = nc.dram_tensor("wv", [D, D], F32, kind="ExternalInput").ap()
    bq_d = nc.dram_tensor("bq", [1, D], F32, kind="ExternalInput").ap()
    bk_d = nc.dram_tensor("bk", [1, D], F32, kind="ExternalInput").ap()
    bv_d = nc.dram_tensor("bv", [1, D], F32, kind="ExternalInput").ap()
    wp_d = nc.dram_tensor("wp", [D, NSH], F32, kind="ExternalInput").ap()
    bp_d = nc.dram_tensor("bp", [1, NSH], F32, kind="ExternalInput").ap()
    out_d = nc.dram_tensor("out", [B, NSH], F32, kind="ExternalOutput").ap()
    if DEBUG_OUTS:
        dbg_r = nc.dram_tensor("dbg_r", [B, L], F32, kind="ExternalOutput").ap()
        dbg_aggt = nc.dram_tensor("dbg_aggt", [128, 16], F32, kind="ExternalOutput").ap()

    with tile.TileContext(nc) as tc:
        with (
            tc.tile_pool(name="cst", bufs=1) as cst,
            tc.tile_pool(name="work", bufs=2) as work,
            tc.tile_pool(name="wpp", bufs=WP_BUFS) as wpp,
            tc.tile_pool(name="outp", bufs=3) as outp,
            tc.tile_pool(name="ps_tp", bufs=2, space="PSUM") as ps_tp,
            tc.tile_pool(name="ps_big", bufs=1, space="PSUM") as ps_big,
            tc.tile_pool(name="ps_out", bufs=4, space="PSUM") as ps_out,
        ):
            # ---------------- constants / weights ----------------
            ident128 = cst.tile([128, 128], F32)
            make_identity(nc, ident128[:, :])
            ident8 = cst.tile([8, 8], F32)
            make_identity(nc, ident8[:, :])
            one1 = cst.tile([1, 1], F32)
            nc.vector.memset(one1[:, :], 1.0)
            ones8f = cst.tile([1, 8], F32)
            nc.vector.memset(ones8f[:, :], 1.0)
            ones8b = cst.tile([1, 8], BF16)
            nc.vector.memset(ones8b[:, :], 1.0)
            # blk3[h, b, m] = SCALE * (m == b): per-batch column selector for the
            # corr reduction over heads (host-provided structural constant)
            blk3_d = nc.dram_tensor("blk3", [8, 8, 8], F32, kind="ExternalInput").ap()
            blk3 = cst.tile([8, 8, 8], F32)
            nc.sync.dma_start(blk3[:, :, :], blk3_d)

            wq_sb = cst.tile([128, 2, 256], F32)
            nc.sync.dma_start(wq_sb[:, :, :], wq_d.rearrange("(c p) d -> p c d", p=128))
            wk_sb = cst.tile([128, 2, 256], F32)
            nc.sync.dma_start(wk_sb[:, :, :], wk_d.rearrange("(c p) d -> p c d", p=128))
            wv_sb = cst.tile([128, 2, 256], F32)
            nc.sync.dma_start(wv_sb[:, :, :], wv_d.rearrange("(c p) d -> p c d", p=128))
            bq_sb = cst.tile([1, 256], F32)
            nc.sync.dma_start(bq_sb[:, :], bq_d)
            bk_sb = cst.tile([1, 256], F32)
            nc.sync.dma_start(bk_sb[:, :], bk_d)
            bv_sb = cst.tile([1, 256], F32)
            nc.sync.dma_start(bv_sb[:, :], bv_d)

            # head-sums of projection weights: WqS[d, h] = sum_z Wq[d, h*32+z]
            wqs = cst.tile([128, 2, 8], F32)
            nc.vector.reduce_sum(out=wqs[:, :, :],
                                 in_=wq_sb[:, :, :].rearrange("p c (h z) -> p c h z", z=DK),
                                 axis=mybir.AxisListType.X)
            wks = cst.tile([128, 2, 8], F32)
            nc.vector.reduce_sum(out=wks[:, :, :],
                                 in_=wk_sb[:, :, :].rearrange("p c (h z) -> p c h z", z=DK),
                                 axis=mybir.AxisListType.X)
            bqs_row = cst.tile([1, 8], F32)
            nc.vector.reduce_sum(out=bqs_row[:, :],
                                 in_=bq_sb[:, :].rearrange("o (h z) -> o h z", z=DK),
                                 axis=mybir.AxisListType.X)
            bks_row = cst.tile([1, 8], F32)
            nc.vector.reduce_sum(out=bks_row[:, :],
                                 in_=bk_sb[:, :].rearrange("o (h z) -> o h z", z=DK),
                                 axis=mybir.AxisListType.X)
            # [1,8] -> [8,1] via K=1 matmul against [1,1] ones
            bqs_ps = ps_tp.tile([8, 1], F32, tag="tp")
            nc.tensor.matmul(bqs_ps[:, :], bqs_row[:, :], one1[:, :], start=True, stop=True)
            bqs_vert = cst.tile([8, 1], F32)
            nc.vector.tensor_copy(bqs_vert[:, :], bqs_ps[:, :])
            bks_ps = ps_tp.tile([8, 1], F32, tag="tp")
            nc.tensor.matmul(bks_ps[:, :], bks_row[:, :], one1[:, :], start=True, stop=True)
            bks_vert = cst.tile([8, 1], F32)
            nc.vector.tensor_copy(bks_vert[:, :], bks_ps[:, :])

            # ---------------- per-batch q/k projections ----------------
            ps_r = ps_big.tile([8, L], F32, tag="big")
            for b in range(B):
                xsT = {}
                for (t_src, w_sum, bias_v, nm) in (
                    (qt_d, wqs, bqs_vert, "q"),
                    (kt_d, wks, bks_vert, "k"),
                ):
                    # host provides x^T per batch: rows [256 b : 256 (b+1)] are [d, l]
                    tr = work.tile([128, 2, L], F32, tag="tr")
                    nc.sync.dma_start(
                        tr[:, :, :],
                        t_src[D * b:D * (b + 1), :].rearrange("(c p) l -> p c l", p=128))
                    # project: xsT[h, l] = sum_d WS[d, h] * xT[d, l]
                    xs = work.tile([8, L], F32, tag=f"{nm}sT")
                    for half in range(2):
                        sl = slice(512 * half, 512 * (half + 1))
                        ps_x = ps_out.tile([8, 512], F32, tag="po")
                        nc.tensor.matmul(ps_x[:, :], w_sum[:, 0, :], tr[:, 0, sl], start=True, stop=False)
                        nc.tensor.matmul(ps_x[:, :], w_sum[:, 1, :], tr[:, 1, sl], start=False, stop=True)
                        # psum->sbuf with per-head bias add
                        nc.vector.tensor_scalar(
                            out=xs[:, sl], in0=ps_x[:, :],
                            scalar1=bias_v[:, 0:1], scalar2=None, op0=mybir.AluOpType.add)
                    xsT[nm] = xs
                # prod_b[h, l] then accumulate into corr rows via blk3 selector
                prod = work.tile([8, L], F32, tag="prod")
                nc.vector.tensor_mul(prod[:, :], xsT["q"][:, :], xsT["k"][:, :])
                for half in range(2):
                    sl = slice(512 * half, 512 * (half + 1))
                    nc.tensor.matmul(ps_r[:, sl], blk3[:, b, :], prod[:, sl],
                                     start=(b == 0), stop=(b == B - 1))

            # ---------------- corr, top-6, softmax, select ----------------
            r_sb = cst.tile([8, L], F32)
            nc.vector.tensor_copy(r_sb[:, :], ps_r[:, :])
            if DEBUG_OUTS:
                nc.sync.dma_start(dbg_r, r_sb[:, :])

            topv = cst.tile([8, 8], F32)
            nc.vector.max(topv[:, :], r_sb[:, :])
            negm0 = cst.tile([8, 1], F32)
            nc.vector.tensor_scalar_mul(negm0[:, :], topv[:, 0:1], -1.0)
            e_sb = cst.tile([8, K_TOP], F32)
            nc.scalar.activation(e_sb[:, :], topv[:, 0:K_TOP],
                                 mybir.ActivationFunctionType.Exp,
                                 bias=negm0[:, 0:1], scale=1.0)
            z_sb = cst.tile([8, 1], F32)
            nc.vector.reduce_sum(out=z_sb[:, :], in_=e_sb[:, :], axis=mybir.AxisListType.X)
            zinv = cst.tile([8, 1], F32)
            nc.vector.reciprocal(zinv[:, :], z_sb[:, :])
            w_sb = cst.tile([8, K_TOP], F32)
            nc.vector.tensor_scalar_mul(w_sb[:, :], e_sb[:, :], zinv[:, 0:1])

            # selu[b, l] = sum_j w_j * (r[b, l] == topv[b, j])
            selu = cst.tile([8, L], F32)
            ohw = cst.tile([8, L], F32)
            for j in range(K_TOP):
                dst = selu if j == 0 else ohw
                nc.vector.tensor_scalar(
                    out=dst[:, :], in0=r_sb[:, :],
                    scalar1=topv[:, j:j + 1], scalar2=w_sb[:, j:j + 1],
                    op0=mybir.AluOpType.is_equal, op1=mybir.AluOpType.mult)
                if j > 0:
                    nc.vector.tensor_add(selu[:, :], selu[:, :], ohw[:, :])

            # transpose sel to [l_local, t] layout (bf16), t = l // 128
            selT = cst.tile([128, 64], F32)
            for t in range(8):
                tp8 = ps_tp.tile([128, 8], F32, tag="tp")
                nc.tensor.transpose(tp8[:, :], selu[0:8, 128 * t:128 * (t + 1)], ident8[:, :])
                nc.vector.tensor_copy(selT[:, 8 * t:8 * (t + 1)], tp8[:, :])

            # vbarT[e, b] = sum_l v[b, l, e] * sel[b, l]
            # DVE: acc[p, e] = sum_t v[b, 128 t + p, e] * sel[b, 128 t + p]
            ones128 = cst.tile([128, 1], F32)
            nc.vector.memset(ones128[:, :], 1.0)
            vbarT = cst.tile([128, 16], F32)
            for b in range(B):
                v_b = work.tile([128, 8, 256], BF16, tag="vb")
                nc.gpsimd.dma_start(
                    v_b[:, :, :],
                    v_d[L * b:L * (b + 1), :].rearrange("(t p) d -> p t d", p=128))
                acc = work.tile([128, 256], F32, tag="acc")
                nc.vector.tensor_scalar(
                    out=acc[:, :], in0=v_b[:, 0, :],
                    scalar1=selT[:, b:b + 1], scalar2=None, op0=mybir.AluOpType.mult)
                for t in range(1, 8):
                    nc.vector.scalar_tensor_tensor(
                        out=acc[:, :], in0=v_b[:, t, :],
                        scalar=selT[:, 8 * t + b:8 * t + b + 1], in1=acc[:, :],
                        op0=mybir.AluOpType.mult, op1=mybir.AluOpType.add)
                for e in range(2):
                    pv = ps_tp.tile([128, 1], F32, tag="tp")
                    nc.tensor.matmul(pv[:, :], acc[:, 128 * e:128 * (e + 1)],
                                     ones128[:, :], start=True, stop=True)
                    nc.vector.tensor_copy(vbarT[:, 8 * e + b:8 * e + b + 1], pv[:, :])

            # aggT[d', b] = sum_e Wv[e, d'] * vbarT[e, b] + bv[d']   (bf16 out)
            aggt_bf = cst.tile([128, 16], BF16)
            for m in range(2):
                ps_a = ps_tp.tile([128, 8], F32, tag="tp")
                nc.tensor.matmul(ps_a[:, :], wv_sb[:, 0, 128 * m:128 * (m + 1)],
                                 vbarT[:, 0:8], start=True, stop=False)
                nc.tensor.matmul(ps_a[:, :], wv_sb[:, 1, 128 * m:128 * (m + 1)],
                                 vbarT[:, 8:16], start=False, stop=False)
                nc.tensor.matmul(ps_a[:, :], bv_sb[0:1, 128 * m:128 * (m + 1)],
                                 ones8f[:, :], start=False, stop=True)
                nc.vector.tensor_copy(aggt_bf[:, 8 * m:8 * (m + 1)], ps_a[:, :])
            if DEBUG_OUTS:
                aggt_f = cst.tile([128, 16], F32)
                nc.vector.tensor_copy(aggt_f[:, :], aggt_bf[:, :])
                nc.sync.dma_start(dbg_aggt, aggt_f[:, :])

            # ---------------- big output projection (column shard) ----------------
            for nt in range(N_TILES):
                ncol = slice(TILE_N * nt, TILE_N * (nt + 1))
                wp0 = wpp.tile([128, TILE_N], BF16, tag="wp0")
                nc.gpsimd.dma_start(wp0[:, :], wp_d[0:128, ncol])
                wp1 = wpp.tile([128, TILE_N], BF16, tag="wp1")
                nc.gpsimd.dma_start(wp1[:, :], wp_d[128:256, ncol])
                bp_t = wpp.tile([1, TILE_N], BF16, tag="bp", bufs=2)
                nc.gpsimd.dma_start(bp_t[:, :], bp_d[0:1, ncol])
                bp_rep = wpp.tile([8, TILE_N], BF16, tag="bprep", bufs=2)
                nc.gpsimd.partition_broadcast(bp_rep[:, :], bp_t[:, :])
                o_sb = outp.tile([8, TILE_N], F32)
                for s in range(SUBS):
                    ssl = slice(512 * s, 512 * (s + 1))
                    ps = ps_out.tile([8, 512], F32, tag="po")
                    nc.tensor.matmul(ps[:, :], aggt_bf[:, 0:8], wp0[:, ssl], start=True, stop=False)
                    nc.tensor.matmul(ps[:, :], aggt_bf[:, 8:16], wp1[:, ssl], start=False, stop=True)
                    if s % 2 == 0:
                        nc.scalar.copy(o_sb[:, ssl], ps[:, :])
                    else:
                        nc.vector.tensor_copy(o_sb[:, ssl], ps[:, :])
                nc.vector.tensor_add(o_sb[:, :], o_sb[:, :], bp_rep[:, :])
                nc.sync.dma_start(out_d[:, ncol], o_sb[:, :])

    nc.finalize()
    return nc


def _build_nc_dp():
    """Batch-parallel variant: core i preprocesses batch i only, then an
    AllGather of the tiny agg vector feeds the column-sharded projection."""
    nc = bacc.Bacc("TRN2", target_bir_lowering=False, debug=False, num_devices=N_CORES)

    q_d = nc.dram_tensor("q", [L, D], F32, kind="ExternalInput").ap()
    k_d = nc.dram_tensor("k", [L, D], F32, kind="ExternalInput").ap()
    v_d = nc.dram_tensor("v", [L, D], F32, kind="ExternalInput").ap()
    wq_d = nc.dram_tensor("wq", [D, D], F32, kind="ExternalInput").ap()
    wk_d = nc.dram_tensor("wk", [D, D], F32, kind="ExternalInput").ap()
    wv_d = nc.dram_tensor("wv", [D, D], F32, kind="ExternalInput").ap()
    bq_d = nc.dram_tensor("bq", [1, D], F32, kind="ExternalInput").ap()
    bk_d = nc.dram_tensor("bk", [1, D], F32, kind="ExternalInput").ap()
    bv_d = nc.dram_tensor("bv", [1, D], F32, kind="ExternalInput").ap()
    wp_d = nc.dram_tensor("wp", [D, NSH], F32, kind="ExternalInput").ap()
    bp_d = nc.dram_tensor("bp", [1, NSH], F32, kind="ExternalInput").ap()
    out_d = nc.dram_tensor("out", [B, NSH], F32, kind="ExternalOutput").ap()

    with tile.TileContext(nc) as tc:
        with (
            tc.tile_pool(name="cst", bufs=1) as cst,
            tc.tile_pool(name="work", bufs=2) as work,
            tc.tile_pool(name="wpp", bufs=WP_BUFS) as wpp,
            tc.tile_pool(name="outp", bufs=3) as outp,
            tc.tile_pool(name="dr", bufs=1, space="DRAM") as dr,
            tc.tile_pool(name="ps_tp", bufs=2, space="PSUM") as ps_tp,
            tc.tile_pool(name="ps_big", bufs=1, space="PSUM") as ps_big,
            tc.tile_pool(name="ps_out", bufs=2, space="PSUM") as ps_out,
        ):
            ident128 = cst.tile([128, 128], F32)
            make_identity(nc, ident128[:, :])
            ident8 = cst.tile([8, 8], F32)
            make_identity(nc, ident8[:, :])
            one1 = cst.tile([1, 1], F32)
            nc.vector.memset(one1[:, :], 1.0)
            ones8b = cst.tile([1, 8], BF16)
            nc.vector.memset(ones8b[:, :], 1.0)
            ones128 = cst.tile([128, 1], F32)
            nc.vector.memset(ones128[:, :], 1.0)

            wq_sb = cst.tile([128, 2, 256], F32)
            nc.sync.dma_start(wq_sb[:, :, :], wq_d.rearrange("(c p) d -> p c d", p=128))
            wk_sb = cst.tile([128, 2, 256], F32)
            nc.sync.dma_start(wk_sb[:, :, :], wk_d.rearrange("(c p) d -> p c d", p=128))
            wv_sb = cst.tile([128, 2, 256], F32)
            nc.sync.dma_start(wv_sb[:, :, :], wv_d.rearrange("(c p) d -> p c d", p=128))
            bq_sb = cst.tile([1, 256], F32)
            nc.sync.dma_start(bq_sb[:, :], bq_d)
            bk_sb = cst.tile([1, 256], F32)
            nc.sync.dma_start(bk_sb[:, :], bk_d)
            bv_sb = cst.tile([1, 256], F32)
            nc.sync.dma_start(bv_sb[:, :], bv_d)

            wqs = cst.tile([128, 2, 8], F32)
            nc.vector.reduce_sum(out=wqs[:, :, :],
                                 in_=wq_sb[:, :, :].rearrange("p c (h z) -> p c h z", z=DK),
                                 axis=mybir.AxisListType.X)
            wks = cst.tile([128, 2, 8], F32)
            nc.vector.reduce_sum(out=wks[:, :, :],
                                 in_=wk_sb[:, :, :].rearrange("p c (h z) -> p c h z", z=DK),
                                 axis=mybir.AxisListType.X)
            bqs_row = cst.tile([1, 8], F32)
            nc.vector.reduce_sum(out=bqs_row[:, :],
                                 in_=bq_sb[:, :].rearrange("o (h z) -> o h z", z=DK),
                                 axis=mybir.AxisListType.X)
            bks_row = cst.tile([1, 8], F32)
            nc.vector.reduce_sum(out=bks_row[:, :],
                                 in_=bk_sb[:, :].rearrange("o (h z) -> o h z", z=DK),
                                 axis=mybir.AxisListType.X)
            bqs_ps = ps_tp.tile([8, 1], F32, tag="tp")
            nc.tensor.matmul(bqs_ps[:, :], bqs_row[:, :], one1[:, :], start=True, stop=True)
            bqs_vert = cst.tile([8, 1], F32)
            nc.vector.tensor_copy(bqs_vert[:, :], bqs_ps[:, :])
            bks_ps = ps_tp.tile([8, 1], F32, tag="tp")
            nc.tensor.matmul(bks_ps[:, :], bks_row[:, :], one1[:, :], start=True, stop=True)
            bks_vert = cst.tile([8, 1], F32)
            nc.vector.tensor_copy(bks_vert[:, :], bks_ps[:, :])

            # this core's batch of v, bf16 (gpsimd cast-DMA; first gpsimd instr)
            v_all = cst.tile([128, 8, 256], BF16)
            nc.gpsimd.dma_start(v_all[:, :, :],
                                v_d.rearrange("(t p) d -> p t d", p=128))

            # ---- single-batch q/k projections ----
            xsT = {}
            for (nat_src, w_sum, bias_v, nm) in (
                (q_d, wqs, bqs_vert, "q"),
                (k_d, wks, bks_vert, "k"),
            ):
                nat = work.tile([128, 8, 256], F32, tag="nat")
                nc.sync.dma_start(nat[:, :, :], nat_src.rearrange("(t p) d -> p t d", p=128))
                tr = work.tile([128, 2, L], F32, tag="tr")
                for t in range(8):
                    for c in range(2):
                        tp = ps_tp.tile([128, 128], F32, tag="tp")
                        nc.tensor.transpose(tp[:, :], nat[:, t, 128 * c:128 * (c + 1)], ident128[:, :])
                        nc.vector.tensor_copy(tr[:, c, 128 * t:128 * (t + 1)], tp[:, :])
                xs = work.tile([8, L], F32, tag=f"{nm}sT")
                for half in range(2):
                    sl = slice(512 * half, 512 * (half + 1))
                    ps_x = ps_out.tile([8, 512], F32, tag="po")
                    nc.tensor.matmul(ps_x[:, :], w_sum[:, 0, :], tr[:, 0, sl], start=True, stop=False)
                    nc.tensor.matmul(ps_x[:, :], w_sum[:, 1, :], tr[:, 1, sl], start=False, stop=True)
                    nc.vector.tensor_scalar(
                        out=xs[:, sl], in0=ps_x[:, :],
                        scalar1=bias_v[:, 0:1], scalar2=None, op0=mybir.AluOpType.add)
                xsT[nm] = xs

            prod = work.tile([8, L], F32, tag="prod")
            nc.vector.tensor_mul(prod[:, :], xsT["q"][:, :], xsT["k"][:, :])
            # corr row: r[0, l] = SCALE * sum_h prod[h, l]
            sones = cst.tile([8, 1], F32)
            nc.vector.memset(sones[:, :], SCALE)
            ps_r = ps_big.tile([1, L], F32, tag="big")
            for half in range(2):
                sl = slice(512 * half, 512 * (half + 1))
                nc.tensor.matmul(ps_r[:, sl], sones[:, :], prod[:, sl], start=True, stop=True)
            r_sb = cst.tile([1, L], F32)
            nc.vector.tensor_copy(r_sb[:, :], ps_r[:, :])

            topv = cst.tile([1, 8], F32)
            nc.vector.max(topv[:, :], r_sb[:, :])
            negm0 = cst.tile([1, 1], F32)
            nc.vector.tensor_scalar_mul(negm0[:, :], topv[:, 0:1], -1.0)
            e_sb = cst.tile([1, K_TOP], F32)
            nc.scalar.activation(e_sb[:, :], topv[:, 0:K_TOP],
                                 mybir.ActivationFunctionType.Exp,
                                 bias=negm0[:, 0:1], scale=1.0)
            z_sb = cst.tile([1, 1], F32)
            nc.vector.reduce_sum(out=z_sb[:, :], in_=e_sb[:, :], axis=mybir.AxisListType.X)
            zinv = cst.tile([1, 1], F32)
            nc.vector.reciprocal(zinv[:, :], z_sb[:, :])
            w_sb = cst.tile([1, K_TOP], F32)
            nc.vector.tensor_scalar_mul(w_sb[:, :], e_sb[:, :], zinv[:, 0:1])

            selu = cst.tile([1, L], F32)
            ohw = cst.tile([1, L], F32)
            for j in range(K_TOP):
                dst = selu if j == 0 else ohw
                nc.vector.tensor_scalar(
                    out=dst[:, :], in0=r_sb[:, :],
                    scalar1=topv[:, j:j + 1], scalar2=w_sb[:, j:j + 1],
                    op0=mybir.AluOpType.is_equal, op1=mybir.AluOpType.mult)
                if j > 0:
                    nc.vector.tensor_add(selu[:, :], selu[:, :], ohw[:, :])

            # selT[p, t] = selu[0, 128 t + p] via K=1 matmuls
            selT = cst.tile([128, 8], F32)
            for t in range(8):
                tps = ps_tp.tile([128, 1], F32, tag="tp")
                nc.tensor.matmul(tps[:, :], selu[0:1, 128 * t:128 * (t + 1)], one1[:, :],
                                 start=True, stop=True)
                nc.vector.tensor_copy(selT[:, t:t + 1], tps[:, :])

            # acc[p, e] = sum_t v[128 t + p, e] * sel[128 t + p]   (DVE)
            acc = cst.tile([128, 256], F32)
            nc.vector.tensor_scalar(out=acc[:, :], in0=v_all[:, 0, :],
                                    scalar1=selT[:, 0:1], scalar2=None,
                                    op0=mybir.AluOpType.mult)
            tmp_ac = cst.tile([128, 256], F32)
            for t in range(1, 8):
                nc.vector.tensor_scalar(out=tmp_ac[:, :], in0=v_all[:, t, :],
                                        scalar1=selT[:, t:t + 1], scalar2=None,
                                        op0=mybir.AluOpType.mult)
                nc.vector.tensor_add(acc[:, :], acc[:, :], tmp_ac[:, :])

            # vbarT[e] = sum_p acc[p, e]  -> [128, 2] (e chunks)
            vbarT = cst.tile([128, 2], F32)
            for m in range(2):
                pv = ps_tp.tile([128, 1], F32, tag="tp")
                nc.tensor.matmul(pv[:, :], acc[:, 128 * m:128 * (m + 1)], ones128[:, :],
                                 start=True, stop=True)
                nc.vector.tensor_copy(vbarT[:, m:m + 1], pv[:, :])

            # agg[d'] = sum_e Wv[e, d'] vbarT[e] + bv[d']  -> [128, 2] (d' chunks)
            agg_sb = cst.tile([128, 2], F32)
            for m in range(2):
                pa = ps_tp.tile([128, 1], F32, tag="tp")
                nc.tensor.matmul(pa[:, :], wv_sb[:, 0, 128 * m:128 * (m + 1)],
                                 vbarT[:, 0:1], start=True, stop=False)
                nc.tensor.matmul(pa[:, :], wv_sb[:, 1, 128 * m:128 * (m + 1)],
                                 vbarT[:, 1:2], start=False, stop=False)
                nc.tensor.matmul(pa[:, :], bv_sb[0:1, 128 * m:128 * (m + 1)],
                                 one1[:, :], start=False, stop=True)
                nc.vector.tensor_copy(agg_sb[:, m:m + 1], pa[:, :])

            # AllGather agg -> [8, 256]
            agg_in = dr.tile([1, D], F32)
            nc.sync.dma_start(
                agg_in[:, :].rearrange("o (m e) -> (o e) m", e=128), agg_sb[:, :])
            agg_out = dr.tile([B, D], F32)
            nc.gpsimd.collective_compute(
                "AllGather", mybir.AluOpType.bypass,
                replica_groups=[list(range(N_CORES))],
                ins=[agg_in[:, :].opt()], outs=[agg_out[:, :].opt()])
            aggf = cst.tile([8, 256], F32)
            nc.sync.dma_start(aggf[:, :], agg_out[:, :])
            aggt_bf = cst.tile([128, 16], BF16)
            for m in range(2):
                pt = ps_tp.tile([128, 8], F32, tag="tp")
                nc.tensor.transpose(pt[:, :], aggf[0:8, 128 * m:128 * (m + 1)], ident8[:, :])
                nc.vector.tensor_copy(aggt_bf[:, 8 * m:8 * (m + 1)], pt[:, :])

            # ---- big output projection; wp loads f32 on sync, DVE casts to bf16 ----
            for nt in range(N_TILES):
                ncol = slice(TILE_N * nt, TILE_N * (nt + 1))
                wp0f = wpp.tile([128, TILE_N], F32, tag="wp0f", bufs=3)
                nc.sync.dma_start(wp0f[:, :], wp_d[0:128, ncol])
                wp1f = wpp.tile([128, TILE_N], F32, tag="wp1f", bufs=3)
                nc.sync.dma_start(wp1f[:, :], wp_d[128:256, ncol])
                wp0 = wpp.tile([128, TILE_N], BF16, tag="wp0")
                nc.vector.tensor_copy(wp0[:, :], wp0f[:, :])
                wp1 = wpp.tile([128, TILE_N], BF16, tag="wp1")
                nc.vector.tensor_copy(wp1[:, :], wp1f[:, :])
                bp_t = wpp.tile([1, TILE_N], BF16, tag="bp", bufs=2)
                nc.gpsimd.dma_start(bp_t[:, :], bp_d[0:1, ncol])
                bp_rep = wpp.tile([8, TILE_N], BF16, tag="bprep", bufs=2)
                nc.gpsimd.partition_broadcast(bp_rep[:, :], bp_t[:, :])
                o_sb = outp.tile([8, TILE_N], F32)
                for s in range(SUBS):
                    ssl = slice(512 * s, 512 * (s + 1))
                    ps = ps_out.tile([8, 512], F32, tag="po")
                    nc.tensor.matmul(ps[:, :], aggt_bf[:, 0:8], wp0[:, ssl], start=True, stop=False)
                    nc.tensor.matmul(ps[:, :], aggt_bf[:, 8:16], wp1[:, ssl], start=False, stop=True)
                    if s % 2 == 0:
                        nc.scalar.copy(o_sb[:, ssl], ps[:, :])
                    else:
                        nc.vector.tensor_copy(o_sb[:, ssl], ps[:, :])
                nc.vector.tensor_add(o_sb[:, :], o_sb[:, :], bp_rep[:, :])
                nc.sync.dma_start(out_d[:, ncol], o_sb[:, :])

    nc.finalize()
    return nc


def _get_nc():
    if "nc" not in _CACHE:
        _CACHE["nc"] = _build_nc_dp() if MODE == "dp" else _build_nc()
    return _CACHE["nc"]


def kernel(queries, keys, values, Wq, bq, Wk, bk, Wv, bv, Wp, bp):
    queries = np.ascontiguousarray(np.asarray(queries, np.float32).reshape(B * L, D))
    keys = np.ascontiguousarray(np.asarray(keys, np.float32).reshape(B * L, D))
    values = np.ascontiguousarray(np.asarray(values, np.float32).reshape(B * L, D))
    Wq = np.ascontiguousarray(np.asarray(Wq, np.float32))
    Wk = np.ascontiguousarray(np.asarray(Wk, np.float32))
    Wv = np.ascontiguousarray(np.asarray(Wv, np.float32))
    bq = np.asarray(bq, np.float32).reshape(1, D)
    bk = np.asarray(bk, np.float32).reshape(1, D)
    bv = np.asarray(bv, np.float32).reshape(1, D)
    Wp = np.asarray(Wp, np.float32)
    bp = np.asarray(bp, np.float32)

    nc = _get_nc()
    qT = np.ascontiguousarray(
        queries.reshape(B, L, D).transpose(0, 2, 1).reshape(B * D, L))
    kT = np.ascontiguousarray(
        keys.reshape(B, L, D).transpose(0, 2, 1).reshape(B * D, L))
    blk3_const = np.zeros((8, 8, 8), np.float32)
    for b in range(B):
        blk3_const[:, b, b] = SCALE
    in_maps = []
    for i in range(N_CORES):
        cols = slice(NSH * i, NSH * (i + 1))
        m = {
            "wq": Wq, "wk": Wk, "wv": Wv,
            "bq": bq, "bk": bk, "bv": bv,
            "wp": np.ascontiguousarray(Wp[:, cols]),
            "bp": np.ascontiguousarray(bp[cols]).reshape(1, NSH),
        }
        if MODE == "dp":
            rows = slice(L * i, L * (i + 1))
            m.update({"q": queries[rows], "k": keys[rows], "v": values[rows]})
        else:
            m.update({"qt": qT, "kt": kT, "v": values, "blk3": blk3_const})
        in_maps.append(m)
    res = run_bass_kernel_spmd(nc, in_maps, core_ids=list(range(N_CORES)), trace=TRACE)
    global LAST_RESULT
    LAST_RESULT = res
    out = np.concatenate([res.results[i]["out"] for i in range(N_CORES)], axis=1)
    return out.reshape(B, L, D)


# revision 17
# speedup vs baseline: 1.0954x; 1.0954x over previous
"""AutoCorrelation layer kernel for 8 Trainium2 NeuronCores.

Math note: the reference's rfft/irfft pair over the zero-padded head dim
computes a circular cross-correlation; its mean over all lags collapses
analytically to (sum_d q_proj) * (sum_d k_proj) per head.  So
corr_mean[b,l] = (1/(H*L)) * sum_h (q[b,l] @ WqS + bqS)_h * (k[b,l] @ WkS + bkS)_h
with WqS = Wq.reshape(D,H,DK).sum(-1).  Everything downstream (top-6,
softmax, gather, output projection) follows the reference directly.

Distribution: all 8 cores redundantly compute the cheap preprocessing
(full q/k/v, ~24MB) and each core computes its own column shard of the
huge (256, 262144) output projection (column-parallel, no collectives).
"""
import sys
import types

sys.path.insert(0, "/opt/trn_rl_repo")

import numpy as np
import concourse.bass as bass
import concourse.mybir as mybir
import concourse.tile as tile
from concourse import bacc
from concourse.bass_utils import run_bass_kernel_spmd
from concourse.masks import make_identity

F32 = mybir.dt.float32
BF16 = mybir.dt.bfloat16

N_CORES = 8
B, L, D, H, DK = 8, 1024, 256, 8, 32
K_TOP = 6
NSH = (L * D) // N_CORES          # 32768 output cols per core
TILE_N = 2048
N_TILES = NSH // TILE_N           # 16
SUBS = TILE_N // 512              # 4
SCALE = 1.0 / (H * L)

WP_BUFS = 6
MODE = "rep"  # "dp" = batch-parallel preproc + AllGather; "rep" = redundant preproc
DEBUG_OUTS = False
TRACE = False          # test harness sets this for profiled runs
LAST_RESULT = None     # stashed BassKernelResults from the last kernel() call

_CACHE = {}


def _build_nc():
    nc = bacc.Bacc("TRN2", target_bir_lowering=False, debug=False, num_devices=N_CORES)

    qt_d = nc.dram_tensor("qt", [B * D, L], F32, kind="ExternalInput").ap()
    kt_d = nc.dram_tensor("kt", [B * D, L], F32, kind="ExternalInput").ap()
    v_d = nc.dram_tensor("v", [B * L, D], F32, kind="ExternalInput").ap()
    wq_d = nc.dram_tensor("wq", [D, D], F32, kind="ExternalInput").ap()
    wk_d = nc.dram_tensor("wk", [D, D], F32, kind="ExternalInput").ap()
    wv_d = nc.dram_tensor("wv", [D, D], F32, kind="ExternalInput").ap()
    bq_d = nc.dram_tensor("bq", [1, D], F32, kind="ExternalInput").ap()
    bk_d = nc.dram_tensor("bk", [1, D], F32, kind="ExternalInput").ap()
    bv_d = nc.dram_tensor("bv", [1, D], F32, kind="ExternalInput").ap()
    wp_d = nc.dram_tensor("wp", [D, NSH], F32, kind="ExternalInput").ap()
    bp_d = nc.dram_tensor("bp", [1, NSH], F32, kind="ExternalInput").ap()
    out_d = nc.dram_tensor("out", [B, NSH], F32, kind="ExternalOutput").ap()
    if DEBUG_OUTS:
        dbg_r = nc.dram_tensor("dbg_r", [B, L], F32, kind="ExternalOutput").ap()
        dbg_aggt = nc.dram_tensor("dbg_aggt", [128, 16], F32, kind="ExternalOutput").ap()

    with tile.TileContext(nc) as tc:
        with (
            tc.tile_pool(name="cst", bufs=1) as cst,
            tc.tile_pool(name="work", bufs=2) as work,
            tc.tile_pool(name="wpp", bufs=WP_BUFS) as wpp,
            tc.tile_pool(name="outp", bufs=3) as outp,
            tc.tile_pool(name="ps_tp", bufs=2, space="PSUM") as ps_tp,
            tc.tile_pool(name="ps_big", bufs=1, space="PSUM") as ps_big,
            tc.tile_pool(name="ps_out", bufs=4, space="PSUM") as ps_out,
        ):
            # ---------------- constants / weights ----------------
            ident128 = cst.tile([128, 128], F32)
            make_identity(nc, ident128[:, :])
            ident8 = cst.tile([8, 8], F32)
            make_identity(nc, ident8[:, :])
            one1 = cst.tile([1, 1], F32)
            nc.vector.memset(one1[:, :], 1.0)
            ones8f = cst.tile([1, 8], F32)
            nc.vector.memset(ones8f[:, :], 1.0)
            ones8b = cst.tile([1, 8], BF16)
            nc.vector.memset(ones8b[:, :], 1.0)
            # blk3[h, b, m] = SCALE * (m == b): per-batch column selector for the
            # corr reduction over heads (host-provided structural constant)
            blk3_d = nc.dram_tensor("blk3", [8, 8, 8], F32, kind="ExternalInput").ap()
            blk3 = cst.tile([8, 8, 8], F32)
            nc.sync.dma_start(blk3[:, :, :], blk3_d)

            wq_sb = cst.tile([128, 2, 256], F32)
            nc.sync.dma_start(wq_sb[:, :, :], wq_d.rearrange("(c p) d -> p c d", p=128))
            wk_sb = cst.tile([128, 2, 256], F32)
            nc.sync.dma_start(wk_sb[:, :, :], wk_d.rearrange("(c p) d -> p c d", p=128))
            wv_sb = cst.tile([128, 2, 256], F32)
            nc.sync.dma_start(wv_sb[:, :, :], wv_d.rearrange("(c p) d -> p c d", p=128))
            bq_sb = cst.tile([1, 256], F32)
            nc.sync.dma_start(bq_sb[:, :], bq_d)
            bk_sb = cst.tile([1, 256], F32)
            nc.sync.dma_start(bk_sb[:, :], bk_d)
            bv_sb = cst.tile([1, 256], F32)
            nc.sync.dma_start(bv_sb[:, :], bv_d)

            # head-sums of projection weights: WqS[d, h] = sum_z Wq[d, h*32+z]
            wqs = cst.tile([128, 2, 8], F32)
            nc.vector.reduce_sum(out=wqs[:, :, :],
                                 in_=wq_sb[:, :, :].rearrange("p c (h z) -> p c h z", z=DK),
                                 axis=mybir.AxisListType.X)
            wks = cst.tile([128, 2, 8], F32)
            nc.vector.reduce_sum(out=wks[:, :, :],
                                 in_=wk_sb[:, :, :].rearrange("p c (h z) -> p c h z", z=DK),
                                 axis=mybir.AxisListType.X)
            bqs_row = cst.tile([1, 8], F32)
            nc.vector.reduce_sum(out=bqs_row[:, :],
                                 in_=bq_sb[:, :].rearrange("o (h z) -> o h z", z=DK),
                                 axis=mybir.AxisListType.X)
            bks_row = cst.tile([1, 8], F32)
            nc.vector.reduce_sum(out=bks_row[:, :],
                                 in_=bk_sb[:, :].rearrange("o (h z) -> o h z", z=DK),
                                 axis=mybir.AxisListType.X)
            # [1,8] -> [8,1] via K=1 matmul against [1,1] ones
            bqs_ps = ps_tp.tile([8, 1], F32, tag="tp")
            nc.tensor.matmul(bqs_ps[:, :], bqs_row[:, :], one1[:, :], start=True, stop=True)
            bqs_vert = cst.tile([8, 1], F32)
            nc.vector.tensor_copy(bqs_vert[:, :], bqs_ps[:, :])
            bks_ps = ps_tp.tile([8, 1], F32, tag="tp")
            nc.tensor.matmul(bks_ps[:, :], bks_row[:, :], one1[:, :], start=True, stop=True)
            bks_vert = cst.tile([8, 1], F32)
            nc.vector.tensor_copy(bks_vert[:, :], bks_ps[:, :])

            # v (bf16, cast in DMA) for the weighted gather
            v_all = cst.tile([128, B, 8, 256], BF16)
            nc.gpsimd.dma_start(v_all[:, :, :, :],
                                v_d.rearrange("(b t p) d -> p b t d", p=128, t=8))

            # ---------------- per-batch q/k projections ----------------
            ps_r = ps_big.tile([8, L], F32, tag="big")
            for b in range(B):
                xsT = {}
                for (t_src, w_sum, bias_v, nm) in (
                    (qt_d, wqs, bqs_vert, "q"),
                    (kt_d, wks, bks_vert, "k"),
                ):
                    # host provides x^T per batch: rows [256 b : 256 (b+1)] are [d, l]
                    tr = work.tile([128, 2, L], F32, tag="tr")
                    nc.sync.dma_start(
                        tr[:, :, :],
                        t_src[D * b:D * (b + 1), :].rearrange("(c p) l -> p c l", p=128))
                    # project: xsT[h, l] = sum_d WS[d, h] * xT[d, l]
                    xs = work.tile([8, L], F32, tag=f"{nm}sT")
                    for half in range(2):
                        sl = slice(512 * half, 512 * (half + 1))
                        ps_x = ps_out.tile([8, 512], F32, tag="po")
                        nc.tensor.matmul(ps_x[:, :], w_sum[:, 0, :], tr[:, 0, sl], start=True, stop=False)
                        nc.tensor.matmul(ps_x[:, :], w_sum[:, 1, :], tr[:, 1, sl], start=False, stop=True)
                        # psum->sbuf with per-head bias add
                        nc.vector.tensor_scalar(
                            out=xs[:, sl], in0=ps_x[:, :],
                            scalar1=bias_v[:, 0:1], scalar2=None, op0=mybir.AluOpType.add)
                    xsT[nm] = xs
                # prod_b[h, l] then accumulate into corr rows via blk3 selector
                prod = work.tile([8, L], F32, tag="prod")
                nc.vector.tensor_mul(prod[:, :], xsT["q"][:, :], xsT["k"][:, :])
                for half in range(2):
                    sl = slice(512 * half, 512 * (half + 1))
                    nc.tensor.matmul(ps_r[:, sl], blk3[:, b, :], prod[:, sl],
                                     start=(b == 0), stop=(b == B - 1))

            # ---------------- corr, top-6, softmax, select ----------------
            r_sb = cst.tile([8, L], F32)
            nc.vector.tensor_copy(r_sb[:, :], ps_r[:, :])
            if DEBUG_OUTS:
                nc.sync.dma_start(dbg_r, r_sb[:, :])

            topv = cst.tile([8, 8], F32)
            nc.vector.max(topv[:, :], r_sb[:, :])
            negm0 = cst.tile([8, 1], F32)
            nc.vector.tensor_scalar_mul(negm0[:, :], topv[:, 0:1], -1.0)
            e_sb = cst.tile([8, K_TOP], F32)
            nc.scalar.activation(e_sb[:, :], topv[:, 0:K_TOP],
                                 mybir.ActivationFunctionType.Exp,
                                 bias=negm0[:, 0:1], scale=1.0)
            z_sb = cst.tile([8, 1], F32)
            nc.vector.reduce_sum(out=z_sb[:, :], in_=e_sb[:, :], axis=mybir.AxisListType.X)
            zinv = cst.tile([8, 1], F32)
            nc.vector.reciprocal(zinv[:, :], z_sb[:, :])
            w_sb = cst.tile([8, K_TOP], F32)
            nc.vector.tensor_scalar_mul(w_sb[:, :], e_sb[:, :], zinv[:, 0:1])

            # selu[b, l] = sum_j w_j * (r[b, l] == topv[b, j])
            selu = cst.tile([8, L], F32)
            ohw = cst.tile([8, L], F32)
            for j in range(K_TOP):
                dst = selu if j == 0 else ohw
                nc.vector.tensor_scalar(
                    out=dst[:, :], in0=r_sb[:, :],
                    scalar1=topv[:, j:j + 1], scalar2=w_sb[:, j:j + 1],
                    op0=mybir.AluOpType.is_equal, op1=mybir.AluOpType.mult)
                if j > 0:
                    nc.vector.tensor_add(selu[:, :], selu[:, :], ohw[:, :])

            # transpose sel to [l_local, t] layout (bf16), t = l // 128
            selT = cst.tile([128, 64], F32)
            for t in range(8):
                tp8 = ps_tp.tile([128, 8], F32, tag="tp")
                nc.tensor.transpose(tp8[:, :], selu[0:8, 128 * t:128 * (t + 1)], ident8[:, :])
                nc.vector.tensor_copy(selT[:, 8 * t:8 * (t + 1)], tp8[:, :])

            # vbarT[e, b] = sum_l v[b, l, e] * sel[b, l]
            # DVE: acc[p, e] = sum_t v[b, 128 t + p, e] * sel[b, 128 t + p]
            ones128 = cst.tile([128, 1], F32)
            nc.vector.memset(ones128[:, :], 1.0)
            vbarT = cst.tile([128, 16], F32)
            for b in range(B):
                acc = work.tile([128, 256], F32, tag="acc")
                nc.vector.tensor_scalar(
                    out=acc[:, :], in0=v_all[:, b, 0, :],
                    scalar1=selT[:, b:b + 1], scalar2=None, op0=mybir.AluOpType.mult)
                for t in range(1, 8):
                    nc.vector.scalar_tensor_tensor(
                        out=acc[:, :], in0=v_all[:, b, t, :],
                        scalar=selT[:, 8 * t + b:8 * t + b + 1], in1=acc[:, :],
                        op0=mybir.AluOpType.mult, op1=mybir.AluOpType.add)
                for e in range(2):
                    pv = ps_tp.tile([128, 1], F32, tag="tp")
                    nc.tensor.matmul(pv[:, :], acc[:, 128 * e:128 * (e + 1)],
                                     ones128[:, :], start=True, stop=True)
                    nc.vector.tensor_copy(vbarT[:, 8 * e + b:8 * e + b + 1], pv[:, :])

            # aggT[d', b] = sum_e Wv[e, d'] * vbarT[e, b] + bv[d']   (bf16 out)
            aggt_bf = cst.tile([128, 16], BF16)
            for m in range(2):
                ps_a = ps_tp.tile([128, 8], F32, tag="tp")
                nc.tensor.matmul(ps_a[:, :], wv_sb[:, 0, 128 * m:128 * (m + 1)],
                                 vbarT[:, 0:8], start=True, stop=False)
                nc.tensor.matmul(ps_a[:, :], wv_sb[:, 1, 128 * m:128 * (m + 1)],
                                 vbarT[:, 8:16], start=False, stop=False)
                nc.tensor.matmul(ps_a[:, :], bv_sb[0:1, 128 * m:128 * (m + 1)],
                                 ones8f[:, :], start=False, stop=True)
                nc.vector.tensor_copy(aggt_bf[:, 8 * m:8 * (m + 1)], ps_a[:, :])
            if DEBUG_OUTS:
                aggt_f = cst.tile([128, 16], F32)
                nc.vector.tensor_copy(aggt_f[:, :], aggt_bf[:, :])
                nc.sync.dma_start(dbg_aggt, aggt_f[:, :])

            # ---------------- big output projection (column shard) ----------------
            for nt in range(N_TILES):
                ncol = slice(TILE_N * nt, TILE_N * (nt + 1))
                wp0 = wpp.tile([128, TILE_N], BF16, tag="wp0")
                nc.gpsimd.dma_start(wp0[:, :], wp_d[0:128, ncol])
                wp1 = wpp.tile([128, TILE_N], BF16, tag="wp1")
                nc.gpsimd.dma_start(wp1[:, :], wp_d[128:256, ncol])
                bp_t = wpp.tile([1, TILE_N], BF16, tag="bp", bufs=2)
                nc.gpsimd.dma_start(bp_t[:, :], bp_d[0:1, ncol])
                bp_rep = wpp.tile([8, TILE_N], BF16, tag="bprep", bufs=2)
                nc.gpsimd.partition_broadcast(bp_rep[:, :], bp_t[:, :])
                o_sb = outp.tile([8, TILE_N], F32)
                for s in range(SUBS):
                    ssl = slice(512 * s, 512 * (s + 1))
                    ps = ps_out.tile([8, 512], F32, tag="po")
                    nc.tensor.matmul(ps[:, :], aggt_bf[:, 0:8], wp0[:, ssl], start=True, stop=False)
                    nc.tensor.matmul(ps[:, :], aggt_bf[:, 8:16], wp1[:, ssl], start=False, stop=True)
                    if s % 2 == 0:
                        nc.scalar.copy(o_sb[:, ssl], ps[:, :])
                    else:
                        nc.vector.tensor_copy(o_sb[:, ssl], ps[:, :])
                nc.vector.tensor_add(o_sb[:, :], o_sb[:, :], bp_rep[:, :])
                nc.sync.dma_start(out_d[:, ncol], o_sb[:, :])

    nc.finalize()
    return nc


def _build_nc_dp():
    """Batch-parallel variant: core i preprocesses batch i only, then an
    AllGather of the tiny agg vector feeds the column-sharded projection."""
    nc = bacc.Bacc("TRN2", target_bir_lowering=False, debug=False, num_devices=N_CORES)

    q_d = nc.dram_tensor("q", [L, D], F32, kind="ExternalInput").ap()
    k_d = nc.dram_tensor("k", [L, D], F32, kind="ExternalInput").ap()
    v_d = nc.dram_tensor("v", [L, D], F32, kind="ExternalInput").ap()
    wq_d = nc.dram_tensor("wq", [D, D], F32, kind="ExternalInput").ap()
    wk_d = nc.dram_tensor("wk", [D, D], F32, kind="ExternalInput").ap()
    wv_d = nc.dram_tensor("wv", [D, D], F32, kind="ExternalInput").ap()
    bq_d = nc.dram_tensor("bq", [1, D], F32, kind="ExternalInput").ap()
    bk_d = nc.dram_tensor("bk", [1, D], F32, kind="ExternalInput").ap()
    bv_d = nc.dram_tensor("bv", [1, D], F32, kind="ExternalInput").ap()
    wp_d = nc.dram_tensor("wp", [D, NSH], F32, kind="ExternalInput").ap()
    bp_d = nc.dram_tensor("bp", [1, NSH], F32, kind="ExternalInput").ap()
    out_d = nc.dram_tensor("out", [B, NSH], F32, kind="ExternalOutput").ap()

    with tile.TileContext(nc) as tc:
        with (
            tc.tile_pool(name="cst", bufs=1) as cst,
            tc.tile_pool(name="work", bufs=2) as work,
            tc.tile_pool(name="wpp", bufs=WP_BUFS) as wpp,
            tc.tile_pool(name="outp", bufs=3) as outp,
            tc.tile_pool(name="dr", bufs=1, space="DRAM") as dr,
            tc.tile_pool(name="ps_tp", bufs=2, space="PSUM") as ps_tp,
            tc.tile_pool(name="ps_big", bufs=1, space="PSUM") as ps_big,
            tc.tile_pool(name="ps_out", bufs=2, space="PSUM") as ps_out,
        ):
            ident128 = cst.tile([128, 128], F32)
            make_identity(nc, ident128[:, :])
            ident8 = cst.tile([8, 8], F32)
            make_identity(nc, ident8[:, :])
            one1 = cst.tile([1, 1], F32)
            nc.vector.memset(one1[:, :], 1.0)
            ones8b = cst.tile([1, 8], BF16)
            nc.vector.memset(ones8b[:, :], 1.0)
            ones128 = cst.tile([128, 1], F32)
            nc.vector.memset(ones128[:, :], 1.0)

            wq_sb = cst.tile([128, 2, 256], F32)
            nc.sync.dma_start(wq_sb[:, :, :], wq_d.rearrange("(c p) d -> p c d", p=128))
            wk_sb = cst.tile([128, 2, 256], F32)
            nc.sync.dma_start(wk_sb[:, :, :], wk_d.rearrange("(c p) d -> p c d", p=128))
            wv_sb = cst.tile([128, 2, 256], F32)
            nc.sync.dma_start(wv_sb[:, :, :], wv_d.rearrange("(c p) d -> p c d", p=128))
            bq_sb = cst.tile([1, 256], F32)
            nc.sync.dma_start(bq_sb[:, :], bq_d)
            bk_sb = cst.tile([1, 256], F32)
            nc.sync.dma_start(bk_sb[:, :], bk_d)
            bv_sb = cst.tile([1, 256], F32)
            nc.sync.dma_start(bv_sb[:, :], bv_d)

            wqs = cst.tile([128, 2, 8], F32)
            nc.vector.reduce_sum(out=wqs[:, :, :],
                                 in_=wq_sb[:, :, :].rearrange("p c (h z) -> p c h z", z=DK),
                                 axis=mybir.AxisListType.X)
            wks = cst.tile([128, 2, 8], F32)
            nc.vector.reduce_sum(out=wks[:, :, :],
                                 in_=wk_sb[:, :, :].rearrange("p c (h z) -> p c h z", z=DK),
                                 axis=mybir.AxisListType.X)
            bqs_row = cst.tile([1, 8], F32)
            nc.vector.reduce_sum(out=bqs_row[:, :],
                                 in_=bq_sb[:, :].rearrange("o (h z) -> o h z", z=DK),
                                 axis=mybir.AxisListType.X)
            bks_row = cst.tile([1, 8], F32)
            nc.vector.reduce_sum(out=bks_row[:, :],
                                 in_=bk_sb[:, :].rearrange("o (h z) -> o h z", z=DK),
                                 axis=mybir.AxisListType.X)
            bqs_ps = ps_tp.tile([8, 1], F32, tag="tp")
            nc.tensor.matmul(bqs_ps[:, :], bqs_row[:, :], one1[:, :], start=True, stop=True)
            bqs_vert = cst.tile([8, 1], F32)
            nc.vector.tensor_copy(bqs_vert[:, :], bqs_ps[:, :])
            bks_ps = ps_tp.tile([8, 1], F32, tag="tp")
            nc.tensor.matmul(bks_ps[:, :], bks_row[:, :], one1[:, :], start=True, stop=True)
            bks_vert = cst.tile([8, 1], F32)
            nc.vector.tensor_copy(bks_vert[:, :], bks_ps[:, :])

            # this core's batch of v, bf16 (gpsimd cast-DMA; first gpsimd instr)
            v_all = cst.tile([128, 8, 256], BF16)
            nc.gpsimd.dma_start(v_all[:, :, :],
                                v_d.rearrange("(t p) d -> p t d", p=128))

            # ---- single-batch q/k projections ----
            xsT = {}
            for (nat_src, w_sum, bias_v, nm) in (
                (q_d, wqs, bqs_vert, "q"),
                (k_d, wks, bks_vert, "k"),
            ):
                nat = work.tile([128, 8, 256], F32, tag="nat")
                nc.sync.dma_start(nat[:, :, :], nat_src.rearrange("(t p) d -> p t d", p=128))
                tr = work.tile([128, 2, L], F32, tag="tr")
                for t in range(8):
                    for c in range(2):
                        tp = ps_tp.tile([128, 128], F32, tag="tp")
                        nc.tensor.transpose(tp[:, :], nat[:, t, 128 * c:128 * (c + 1)], ident128[:, :])
                        nc.vector.tensor_copy(tr[:, c, 128 * t:128 * (t + 1)], tp[:, :])
                xs = work.tile([8, L], F32, tag=f"{nm}sT")
                for half in range(2):
                    sl = slice(512 * half, 512 * (half + 1))
                    ps_x = ps_out.tile([8, 512], F32, tag="po")
                    nc.tensor.matmul(ps_x[:, :], w_sum[:, 0, :], tr[:, 0, sl], start=True, stop=False)
                    nc.tensor.matmul(ps_x[:, :], w_sum[:, 1, :], tr[:, 1, sl], start=False, stop=True)
                    nc.vector.tensor_scalar(
                        out=xs[:, sl], in0=ps_x[:, :],
                        scalar1=bias_v[:, 0:1], scalar2=None, op0=mybir.AluOpType.add)
                xsT[nm] = xs

            prod = work.tile([8, L], F32, tag="prod")
            nc.vector.tensor_mul(prod[:, :], xsT["q"][:, :], xsT["k"][:, :])
            # corr row: r[0, l] = SCALE * sum_h prod[h, l]
            sones = cst.tile([8, 1], F32)
            nc.vector.memset(sones[:, :], SCALE)
            ps_r = ps_big.tile([1, L], F32, tag="big")
            for half in range(2):
                sl = slice(512 * half, 512 * (half + 1))
                nc.tensor.matmul(ps_r[:, sl], sones[:, :], prod[:, sl], start=True, stop=True)
            r_sb = cst.tile([1, L], F32)
            nc.vector.tensor_copy(r_sb[:, :], ps_r[:, :])

            topv = cst.tile([1, 8], F32)
            nc.vector.max(topv[:, :], r_sb[:, :])
            negm0 = cst.tile([1, 1], F32)
            nc.vector.tensor_scalar_mul(negm0[:, :], topv[:, 0:1], -1.0)
            e_sb = cst.tile([1, K_TOP], F32)
            nc.scalar.activation(e_sb[:, :], topv[:, 0:K_TOP],
                                 mybir.ActivationFunctionType.Exp,
                                 bias=negm0[:, 0:1], scale=1.0)
            z_sb = cst.tile([1, 1], F32)
            nc.vector.reduce_sum(out=z_sb[:, :], in_=e_sb[:, :], axis=mybir.AxisListType.X)
            zinv = cst.tile([1, 1], F32)
            nc.vector.reciprocal(zinv[:, :], z_sb[:, :])
            w_sb = cst.tile([1, K_TOP], F32)
            nc.vector.tensor_scalar_mul(w_sb[:, :], e_sb[:, :], zinv[:, 0:1])

            selu = cst.tile([1, L], F32)
            ohw = cst.tile([1, L], F32)
            for j in range(K_TOP):
                dst = selu if j == 0 else ohw
                nc.vector.tensor_scalar(
                    out=dst[:, :], in0=r_sb[:, :],
                    scalar1=topv[:, j:j + 1], scalar2=w_sb[:, j:j + 1],
                    op0=mybir.AluOpType.is_equal, op1=mybir.AluOpType.mult)
                if j > 0:
                    nc.vector.tensor_add(selu[:, :], selu[:, :], ohw[:, :])

            # selT[p, t] = selu[0, 128 t + p] via K=1 matmuls
            selT = cst.tile([128, 8], F32)
            for t in range(8):
                tps = ps_tp.tile([128, 1], F32, tag="tp")
                nc.tensor.matmul(tps[:, :], selu[0:1, 128 * t:128 * (t + 1)], one1[:, :],
                                 start=True, stop=True)
                nc.vector.tensor_copy(selT[:, t:t + 1], tps[:, :])

            # acc[p, e] = sum_t v[128 t + p, e] * sel[128 t + p]   (DVE)
            acc = cst.tile([128, 256], F32)
            nc.vector.tensor_scalar(out=acc[:, :], in0=v_all[:, 0, :],
                                    scalar1=selT[:, 0:1], scalar2=None,
                                    op0=mybir.AluOpType.mult)
            tmp_ac = cst.tile([128, 256], F32)
            for t in range(1, 8):
                nc.vector.tensor_scalar(out=tmp_ac[:, :], in0=v_all[:, t, :],
                                        scalar1=selT[:, t:t + 1], scalar2=None,
                                        op0=mybir.AluOpType.mult)
                nc.vector.tensor_add(acc[:, :], acc[:, :], tmp_ac[:, :])

            # vbarT[e] = sum_p acc[p, e]  -> [128, 2] (e chunks)
            vbarT = cst.tile([128, 2], F32)
            for m in range(2):
                pv = ps_tp.tile([128, 1], F32, tag="tp")
                nc.tensor.matmul(pv[:, :], acc[:, 128 * m:128 * (m + 1)], ones128[:, :],
                                 start=True, stop=True)
                nc.vector.tensor_copy(vbarT[:, m:m + 1], pv[:, :])

            # agg[d'] = sum_e Wv[e, d'] vbarT[e] + bv[d']  -> [128, 2] (d' chunks)
            agg_sb = cst.tile([128, 2], F32)
            for m in range(2):
                pa = ps_tp.tile([128, 1], F32, tag="tp")
                nc.tensor.matmul(pa[:, :], wv_sb[:, 0, 128 * m:128 * (m + 1)],
                                 vbarT[:, 0:1], start=True, stop=False)
                nc.tensor.matmul(pa[:, :], wv_sb[:, 1, 128 * m:128 * (m + 1)],
                                 vbarT[:, 1:2], start=False, stop=False)
                nc.tensor.matmul(pa[:, :], bv_sb[0:1, 128 * m:128 * (m + 1)],
                                 one1[:, :], start=False, stop=True)
                nc.vector.tensor_copy(agg_sb[:, m:m + 1], pa[:, :])

            # AllGather agg -> [8, 256]
            agg_in = dr.tile([1, D], F32)
            nc.sync.dma_start(
                agg_in[:, :].rearrange("o (m e) -> (o e) m", e=128), agg_sb[:, :])
            agg_out = dr.tile([B, D], F32)
            nc.gpsimd.collective_compute(
                "AllGather", mybir.AluOpType.bypass,
                replica_groups=[list(range(N_CORES))],
                ins=[agg_in[:, :].opt()], outs=[agg_out[:, :].opt()])
            aggf = cst.tile([8, 256], F32)
            nc.sync.dma_start(aggf[:, :], agg_out[:, :])
            aggt_bf = cst.tile([128, 16], BF16)
            for m in range(2):
                pt = ps_tp.tile([128, 8], F32, tag="tp")
                nc.tensor.transpose(pt[:, :], aggf[0:8, 128 * m:128 * (m + 1)], ident8[:, :])
                nc.vector.tensor_copy(aggt_bf[:, 8 * m:8 * (m + 1)], pt[:, :])

            # ---- big output projection; wp loads f32 on sync, DVE casts to bf16 ----
            for nt in range(N_TILES):
                ncol = slice(TILE_N * nt, TILE_N * (nt + 1))
                wp0f = wpp.tile([128, TILE_N], F32, tag="wp0f", bufs=3)
                nc.sync.dma_start(wp0f[:, :], wp_d[0:128, ncol])
                wp1f = wpp.tile([128, TILE_N], F32, tag="wp1f", bufs=3)
                nc.sync.dma_start(wp1f[:, :], wp_d[128:256, ncol])
                wp0 = wpp.tile([128, TILE_N], BF16, tag="wp0")
                nc.vector.tensor_copy(wp0[:, :], wp0f[:, :])
                wp1 = wpp.tile([128, TILE_N], BF16, tag="wp1")
                nc.vector.tensor_copy(wp1[:, :], wp1f[:, :])
                bp_t = wpp.tile([1, TILE_N], BF16, tag="bp", bufs=2)
                nc.gpsimd.dma_start(bp_t[:, :], bp_d[0:1, ncol])
                bp_rep = wpp.tile([8, TILE_N], BF16, tag="bprep", bufs=2)
                nc.gpsimd.partition_broadcast(bp_rep[:, :], bp_t[:, :])
                o_sb = outp.tile([8, TILE_N], F32)
                for s in range(SUBS):
                    ssl = slice(512 * s, 512 * (s + 1))
                    ps = ps_out.tile([8, 512], F32, tag="po")
                    nc.tensor.matmul(ps[:, :], aggt_bf[:, 0:8], wp0[:, ssl], start=True, stop=False)
                    nc.tensor.matmul(ps[:, :], aggt_bf[:, 8:16], wp1[:, ssl], start=False, stop=True)
                    if s % 2 == 0:
                        nc.scalar.copy(o_sb[:, ssl], ps[:, :])
                    else:
                        nc.vector.tensor_copy(o_sb[:, ssl], ps[:, :])
                nc.vector.tensor_add(o_sb[:, :], o_sb[:, :], bp_rep[:, :])
                nc.sync.dma_start(out_d[:, ncol], o_sb[:, :])

    nc.finalize()
    return nc


def _get_nc():
    if "nc" not in _CACHE:
        _CACHE["nc"] = _build_nc_dp() if MODE == "dp" else _build_nc()
    return _CACHE["nc"]


def kernel(queries, keys, values, Wq, bq, Wk, bk, Wv, bv, Wp, bp):
    queries = np.ascontiguousarray(np.asarray(queries, np.float32).reshape(B * L, D))
    keys = np.ascontiguousarray(np.asarray(keys, np.float32).reshape(B * L, D))
    values = np.ascontiguousarray(np.asarray(values, np.float32).reshape(B * L, D))
    Wq = np.ascontiguousarray(np.asarray(Wq, np.float32))
    Wk = np.ascontiguousarray(np.asarray(Wk, np.float32))
    Wv = np.ascontiguousarray(np.asarray(Wv, np.float32))
    bq = np.asarray(bq, np.float32).reshape(1, D)
    bk = np.asarray(bk, np.float32).reshape(1, D)
    bv = np.asarray(bv, np.float32).reshape(1, D)
    Wp = np.asarray(Wp, np.float32)
    bp = np.asarray(bp, np.float32)

    nc = _get_nc()
    qT = np.ascontiguousarray(
        queries.reshape(B, L, D).transpose(0, 2, 1).reshape(B * D, L))
    kT = np.ascontiguousarray(
        keys.reshape(B, L, D).transpose(0, 2, 1).reshape(B * D, L))
    blk3_const = np.zeros((8, 8, 8), np.float32)
    for b in range(B):
        blk3_const[:, b, b] = SCALE
    in_maps = []
    for i in range(N_CORES):
        cols = slice(NSH * i, NSH * (i + 1))
        m = {
            "wq": Wq, "wk": Wk, "wv": Wv,
            "bq": bq, "bk": bk, "bv": bv,
            "wp": np.ascontiguousarray(Wp[:, cols]),
            "bp": np.ascontiguousarray(bp[cols]).reshape(1, NSH),
        }
        if MODE == "dp":
            rows = slice(L * i, L * (i + 1))
            m.update({"q": queries[rows], "k": keys[rows], "v": values[rows]})
        else:
            m.update({"qt": qT, "kt": kT, "v": values, "blk3": blk3_const})
        in_maps.append(m)
    res = run_bass_kernel_spmd(nc, in_maps, core_ids=list(range(N_CORES)), trace=TRACE)
    global LAST_RESULT
    LAST_RESULT = res
    out = np.concatenate([res.results[i]["out"] for i in range(N_CORES)], axis=1)
    return out.reshape(B, L, D)


# revision 18
# speedup vs baseline: 1.1990x; 1.0946x over previous
"""AutoCorrelation layer kernel for 8 Trainium2 NeuronCores.

Math note: the reference's rfft/irfft pair over the zero-padded head dim
computes a circular cross-correlation; its mean over all lags collapses
analytically to (sum_d q_proj) * (sum_d k_proj) per head.  So
corr_mean[b,l] = (1/(H*L)) * sum_h (q[b,l] @ WqS + bqS)_h * (k[b,l] @ WkS + bkS)_h
with WqS = Wq.reshape(D,H,DK).sum(-1).  Everything downstream (top-6,
softmax, gather, output projection) follows the reference directly.

Distribution: all 8 cores redundantly compute the cheap preprocessing
(full q/k/v, ~24MB) and each core computes its own column shard of the
huge (256, 262144) output projection (column-parallel, no collectives).
"""
import sys
import types

sys.path.insert(0, "/opt/trn_rl_repo")

import numpy as np
import concourse.bass as bass
import concourse.mybir as mybir
import concourse.tile as tile
from concourse import bacc
from concourse.bass_utils import run_bass_kernel_spmd
from concourse.masks import make_identity

F32 = mybir.dt.float32
BF16 = mybir.dt.bfloat16

N_CORES = 8
B, L, D, H, DK = 8, 1024, 256, 8, 32
K_TOP = 6
NSH = (L * D) // N_CORES          # 32768 output cols per core
TILE_N = 2048
N_TILES = NSH // TILE_N           # 16
SUBS = TILE_N // 512              # 4
SCALE = 1.0 / (H * L)

WP_BUFS = 6
MODE = "rep"  # "dp" = batch-parallel preproc + AllGather; "rep" = redundant preproc
DEBUG_OUTS = False
TRACE = False          # test harness sets this for profiled runs
LAST_RESULT = None     # stashed BassKernelResults from the last kernel() call

_CACHE = {}


def _build_nc():
    nc = bacc.Bacc("TRN2", target_bir_lowering=False, debug=False, num_devices=N_CORES)

    qt_d = nc.dram_tensor("qt", [B * D, L], F32, kind="ExternalInput").ap()
    kt_d = nc.dram_tensor("kt", [B * D, L], F32, kind="ExternalInput").ap()
    v_d = nc.dram_tensor("v", [B * L, D], F32, kind="ExternalInput").ap()
    wq_d = nc.dram_tensor("wq", [D, D], F32, kind="ExternalInput").ap()
    wk_d = nc.dram_tensor("wk", [D, D], F32, kind="ExternalInput").ap()
    wv_d = nc.dram_tensor("wv", [D, D], F32, kind="ExternalInput").ap()
    bq_d = nc.dram_tensor("bq", [1, D], F32, kind="ExternalInput").ap()
    bk_d = nc.dram_tensor("bk", [1, D], F32, kind="ExternalInput").ap()
    bv_d = nc.dram_tensor("bv", [1, D], F32, kind="ExternalInput").ap()
    wp_d = nc.dram_tensor("wp", [D, NSH], F32, kind="ExternalInput").ap()
    bp8_d = nc.dram_tensor("bp8", [B, NSH], F32, kind="ExternalInput").ap()
    out_d = nc.dram_tensor("out", [B, NSH], F32, kind="ExternalOutput").ap()
    if DEBUG_OUTS:
        dbg_r = nc.dram_tensor("dbg_r", [B, L], F32, kind="ExternalOutput").ap()
        dbg_aggt = nc.dram_tensor("dbg_aggt", [128, 16], F32, kind="ExternalOutput").ap()

    with tile.TileContext(nc) as tc:
        with (
            tc.tile_pool(name="cst", bufs=1) as cst,
            tc.tile_pool(name="work", bufs=2) as work,
            tc.tile_pool(name="wpp", bufs=WP_BUFS) as wpp,
            tc.tile_pool(name="outp", bufs=3) as outp,
            tc.tile_pool(name="ps_tp", bufs=2, space="PSUM") as ps_tp,
            tc.tile_pool(name="ps_big", bufs=1, space="PSUM") as ps_big,
            tc.tile_pool(name="ps_out", bufs=4, space="PSUM") as ps_out,
        ):
            # ---------------- constants / weights ----------------
            ident128 = cst.tile([128, 128], F32)
            make_identity(nc, ident128[:, :])
            ident8 = cst.tile([8, 8], F32)
            make_identity(nc, ident8[:, :])
            one1 = cst.tile([1, 1], F32)
            nc.vector.memset(one1[:, :], 1.0)
            ones8f = cst.tile([1, 8], F32)
            nc.vector.memset(ones8f[:, :], 1.0)
            ones8b = cst.tile([1, 8], BF16)
            nc.vector.memset(ones8b[:, :], 1.0)
            # blk3[h, b, m] = SCALE * (m == b): per-batch column selector for the
            # corr reduction over heads (host-provided structural constant)
            blk3_d = nc.dram_tensor("blk3", [8, 8, 8], F32, kind="ExternalInput").ap()
            blk3 = cst.tile([8, 8, 8], F32)
            nc.sync.dma_start(blk3[:, :, :], blk3_d)

            wq_sb = cst.tile([128, 2, 256], F32)
            nc.sync.dma_start(wq_sb[:, :, :], wq_d.rearrange("(c p) d -> p c d", p=128))
            wk_sb = cst.tile([128, 2, 256], F32)
            nc.sync.dma_start(wk_sb[:, :, :], wk_d.rearrange("(c p) d -> p c d", p=128))
            wv_sb = cst.tile([128, 2, 256], F32)
            nc.sync.dma_start(wv_sb[:, :, :], wv_d.rearrange("(c p) d -> p c d", p=128))
            bq_sb = cst.tile([1, 256], F32)
            nc.sync.dma_start(bq_sb[:, :], bq_d)
            bk_sb = cst.tile([1, 256], F32)
            nc.sync.dma_start(bk_sb[:, :], bk_d)
            bv_sb = cst.tile([1, 256], F32)
            nc.sync.dma_start(bv_sb[:, :], bv_d)

            # head-sums of projection weights: WqS[d, h] = sum_z Wq[d, h*32+z]
            wqs = cst.tile([128, 2, 8], F32)
            nc.vector.reduce_sum(out=wqs[:, :, :],
                                 in_=wq_sb[:, :, :].rearrange("p c (h z) -> p c h z", z=DK),
                                 axis=mybir.AxisListType.X)
            wks = cst.tile([128, 2, 8], F32)
            nc.vector.reduce_sum(out=wks[:, :, :],
                                 in_=wk_sb[:, :, :].rearrange("p c (h z) -> p c h z", z=DK),
                                 axis=mybir.AxisListType.X)
            bqs_row = cst.tile([1, 8], F32)
            nc.vector.reduce_sum(out=bqs_row[:, :],
                                 in_=bq_sb[:, :].rearrange("o (h z) -> o h z", z=DK),
                                 axis=mybir.AxisListType.X)
            bks_row = cst.tile([1, 8], F32)
            nc.vector.reduce_sum(out=bks_row[:, :],
                                 in_=bk_sb[:, :].rearrange("o (h z) -> o h z", z=DK),
                                 axis=mybir.AxisListType.X)
            # [1,8] -> [8,1] via K=1 matmul against [1,1] ones
            bqs_ps = ps_tp.tile([8, 1], F32, tag="tp")
            nc.tensor.matmul(bqs_ps[:, :], bqs_row[:, :], one1[:, :], start=True, stop=True)
            bqs_vert = cst.tile([8, 1], F32)
            nc.vector.tensor_copy(bqs_vert[:, :], bqs_ps[:, :])
            bks_ps = ps_tp.tile([8, 1], F32, tag="tp")
            nc.tensor.matmul(bks_ps[:, :], bks_row[:, :], one1[:, :], start=True, stop=True)
            bks_vert = cst.tile([8, 1], F32)
            nc.vector.tensor_copy(bks_vert[:, :], bks_ps[:, :])

            # v (bf16, cast in DMA) for the weighted gather
            v_all = cst.tile([128, B, 8, 256], BF16)
            nc.gpsimd.dma_start(v_all[:, :, :, :],
                                v_d.rearrange("(b t p) d -> p b t d", p=128, t=8))

            # ---------------- per-batch q/k projections ----------------
            ps_r = ps_big.tile([8, L], F32, tag="big")
            for b in range(B):
                xsT = {}
                for (t_src, w_sum, bias_v, nm) in (
                    (qt_d, wqs, bqs_vert, "q"),
                    (kt_d, wks, bks_vert, "k"),
                ):
                    # host provides x^T per batch: rows [256 b : 256 (b+1)] are [d, l]
                    tr = work.tile([128, 2, L], F32, tag="tr")
                    nc.sync.dma_start(
                        tr[:, :, :],
                        t_src[D * b:D * (b + 1), :].rearrange("(c p) l -> p c l", p=128))
                    # project: xsT[h, l] = sum_d WS[d, h] * xT[d, l]
                    xs = work.tile([8, L], F32, tag=f"{nm}sT")
                    for half in range(2):
                        sl = slice(512 * half, 512 * (half + 1))
                        ps_x = ps_out.tile([8, 512], F32, tag="po")
                        nc.tensor.matmul(ps_x[:, :], w_sum[:, 0, :], tr[:, 0, sl], start=True, stop=False)
                        nc.tensor.matmul(ps_x[:, :], w_sum[:, 1, :], tr[:, 1, sl], start=False, stop=True)
                        # psum->sbuf with per-head bias add
                        nc.vector.tensor_scalar(
                            out=xs[:, sl], in0=ps_x[:, :],
                            scalar1=bias_v[:, 0:1], scalar2=None, op0=mybir.AluOpType.add)
                    xsT[nm] = xs
                # prod_b[h, l] then accumulate into corr rows via blk3 selector
                prod = work.tile([8, L], F32, tag="prod")
                nc.vector.tensor_mul(prod[:, :], xsT["q"][:, :], xsT["k"][:, :])
                for half in range(2):
                    sl = slice(512 * half, 512 * (half + 1))
                    nc.tensor.matmul(ps_r[:, sl], blk3[:, b, :], prod[:, sl],
                                     start=(b == 0), stop=(b == B - 1))

            # ---------------- corr, top-6, softmax, select ----------------
            r_sb = cst.tile([8, L], F32)
            nc.vector.tensor_copy(r_sb[:, :], ps_r[:, :])
            if DEBUG_OUTS:
                nc.sync.dma_start(dbg_r, r_sb[:, :])

            topv = cst.tile([8, 8], F32)
            nc.vector.max(topv[:, :], r_sb[:, :])
            negm0 = cst.tile([8, 1], F32)
            nc.vector.tensor_scalar_mul(negm0[:, :], topv[:, 0:1], -1.0)
            e_sb = cst.tile([8, K_TOP], F32)
            nc.scalar.activation(e_sb[:, :], topv[:, 0:K_TOP],
                                 mybir.ActivationFunctionType.Exp,
                                 bias=negm0[:, 0:1], scale=1.0)
            z_sb = cst.tile([8, 1], F32)
            nc.vector.reduce_sum(out=z_sb[:, :], in_=e_sb[:, :], axis=mybir.AxisListType.X)
            zinv = cst.tile([8, 1], F32)
            nc.vector.reciprocal(zinv[:, :], z_sb[:, :])
            w_sb = cst.tile([8, K_TOP], F32)
            nc.vector.tensor_scalar_mul(w_sb[:, :], e_sb[:, :], zinv[:, 0:1])

            # selu[b, l] = sum_j w_j * (r[b, l] == topv[b, j])
            selu = cst.tile([8, L], F32)
            ohw = cst.tile([8, L], F32)
            for j in range(K_TOP):
                dst = selu if j == 0 else ohw
                nc.vector.tensor_scalar(
                    out=dst[:, :], in0=r_sb[:, :],
                    scalar1=topv[:, j:j + 1], scalar2=w_sb[:, j:j + 1],
                    op0=mybir.AluOpType.is_equal, op1=mybir.AluOpType.mult)
                if j > 0:
                    nc.vector.tensor_add(selu[:, :], selu[:, :], ohw[:, :])

            # transpose sel to [l_local, t] layout (bf16), t = l // 128
            selT = cst.tile([128, 64], BF16)
            for t in range(8):
                tp8 = ps_tp.tile([128, 8], F32, tag="tp")
                nc.tensor.transpose(tp8[:, :], selu[0:8, 128 * t:128 * (t + 1)], ident8[:, :])
                nc.vector.tensor_copy(selT[:, 8 * t:8 * (t + 1)], tp8[:, :])

            # vbarT[e, b] = sum_l v[b, l, e] * sel[b, l]
            # DVE: acc[p, e] = sum_t v[b, 128 t + p, e] * sel[b, 128 t + p]
            vbarT = cst.tile([128, 16], F32)
            for b in range(B):
                for e in range(2):
                    pv = ps_tp.tile([128, 1], F32, tag="tp")
                    for t in range(8):
                        nc.tensor.matmul(pv[:, :],
                                         v_all[:, b, t, 128 * e:128 * (e + 1)],
                                         selT[:, 8 * t + b:8 * t + b + 1],
                                         start=(t == 0), stop=(t == 7))
                    nc.vector.tensor_copy(vbarT[:, 8 * e + b:8 * e + b + 1], pv[:, :])

            # aggT[d', b] = sum_e Wv[e, d'] * vbarT[e, b] + bv[d']   (bf16 out)
            aggt_bf = cst.tile([128, 16], BF16)
            for m in range(2):
                ps_a = ps_tp.tile([128, 8], F32, tag="tp")
                nc.tensor.matmul(ps_a[:, :], wv_sb[:, 0, 128 * m:128 * (m + 1)],
                                 vbarT[:, 0:8], start=True, stop=False)
                nc.tensor.matmul(ps_a[:, :], wv_sb[:, 1, 128 * m:128 * (m + 1)],
                                 vbarT[:, 8:16], start=False, stop=False)
                nc.tensor.matmul(ps_a[:, :], bv_sb[0:1, 128 * m:128 * (m + 1)],
                                 ones8f[:, :], start=False, stop=True)
                nc.vector.tensor_copy(aggt_bf[:, 8 * m:8 * (m + 1)], ps_a[:, :])
            if DEBUG_OUTS:
                aggt_f = cst.tile([128, 16], F32)
                nc.vector.tensor_copy(aggt_f[:, :], aggt_bf[:, :])
                nc.sync.dma_start(dbg_aggt, aggt_f[:, :])

            # ---------------- big output projection (column shard) ----------------
            for nt in range(N_TILES):
                ncol = slice(TILE_N * nt, TILE_N * (nt + 1))
                wp0 = wpp.tile([128, TILE_N], BF16, tag="wp0")
                nc.gpsimd.dma_start(wp0[:, :], wp_d[0:128, ncol])
                wp1 = wpp.tile([128, TILE_N], BF16, tag="wp1")
                nc.gpsimd.dma_start(wp1[:, :], wp_d[128:256, ncol])
                bp_rep = wpp.tile([8, TILE_N], F32, tag="bprep", bufs=2)
                nc.sync.dma_start(bp_rep[:, :], bp8_d[:, ncol])
                o_sb = outp.tile([8, TILE_N], F32)
                for s in range(SUBS):
                    ssl = slice(512 * s, 512 * (s + 1))
                    ps = ps_out.tile([8, 512], F32, tag="po")
                    nc.tensor.matmul(ps[:, :], aggt_bf[:, 0:8], wp0[:, ssl], start=True, stop=False)
                    nc.tensor.matmul(ps[:, :], aggt_bf[:, 8:16], wp1[:, ssl], start=False, stop=True)
                    if s % 2 == 0:
                        nc.scalar.copy(o_sb[:, ssl], ps[:, :])
                    else:
                        nc.vector.tensor_copy(o_sb[:, ssl], ps[:, :])
                nc.vector.tensor_add(o_sb[:, :], o_sb[:, :], bp_rep[:, :])
                nc.sync.dma_start(out_d[:, ncol], o_sb[:, :])

    nc.finalize()
    return nc


def _build_nc_dp():
    """Batch-parallel variant: core i preprocesses batch i only, then an
    AllGather of the tiny agg vector feeds the column-sharded projection."""
    nc = bacc.Bacc("TRN2", target_bir_lowering=False, debug=False, num_devices=N_CORES)

    q_d = nc.dram_tensor("q", [L, D], F32, kind="ExternalInput").ap()
    k_d = nc.dram_tensor("k", [L, D], F32, kind="ExternalInput").ap()
    v_d = nc.dram_tensor("v", [L, D], F32, kind="ExternalInput").ap()
    wq_d = nc.dram_tensor("wq", [D, D], F32, kind="ExternalInput").ap()
    wk_d = nc.dram_tensor("wk", [D, D], F32, kind="ExternalInput").ap()
    wv_d = nc.dram_tensor("wv", [D, D], F32, kind="ExternalInput").ap()
    bq_d = nc.dram_tensor("bq", [1, D], F32, kind="ExternalInput").ap()
    bk_d = nc.dram_tensor("bk", [1, D], F32, kind="ExternalInput").ap()
    bv_d = nc.dram_tensor("bv", [1, D], F32, kind="ExternalInput").ap()
    wp_d = nc.dram_tensor("wp", [D, NSH], F32, kind="ExternalInput").ap()
    bp_d = nc.dram_tensor("bp", [1, NSH], F32, kind="ExternalInput").ap()
    out_d = nc.dram_tensor("out", [B, NSH], F32, kind="ExternalOutput").ap()

    with tile.TileContext(nc) as tc:
        with (
            tc.tile_pool(name="cst", bufs=1) as cst,
            tc.tile_pool(name="work", bufs=2) as work,
            tc.tile_pool(name="wpp", bufs=WP_BUFS) as wpp,
            tc.tile_pool(name="outp", bufs=3) as outp,
            tc.tile_pool(name="dr", bufs=1, space="DRAM") as dr,
            tc.tile_pool(name="ps_tp", bufs=2, space="PSUM") as ps_tp,
            tc.tile_pool(name="ps_big", bufs=1, space="PSUM") as ps_big,
            tc.tile_pool(name="ps_out", bufs=2, space="PSUM") as ps_out,
        ):
            ident128 = cst.tile([128, 128], F32)
            make_identity(nc, ident128[:, :])
            ident8 = cst.tile([8, 8], F32)
            make_identity(nc, ident8[:, :])
            one1 = cst.tile([1, 1], F32)
            nc.vector.memset(one1[:, :], 1.0)
            ones8b = cst.tile([1, 8], BF16)
            nc.vector.memset(ones8b[:, :], 1.0)
            ones128 = cst.tile([128, 1], F32)
            nc.vector.memset(ones128[:, :], 1.0)

            wq_sb = cst.tile([128, 2, 256], F32)
            nc.sync.dma_start(wq_sb[:, :, :], wq_d.rearrange("(c p) d -> p c d", p=128))
            wk_sb = cst.tile([128, 2, 256], F32)
            nc.sync.dma_start(wk_sb[:, :, :], wk_d.rearrange("(c p) d -> p c d", p=128))
            wv_sb = cst.tile([128, 2, 256], F32)
            nc.sync.dma_start(wv_sb[:, :, :], wv_d.rearrange("(c p) d -> p c d", p=128))
            bq_sb = cst.tile([1, 256], F32)
            nc.sync.dma_start(bq_sb[:, :], bq_d)
            bk_sb = cst.tile([1, 256], F32)
            nc.sync.dma_start(bk_sb[:, :], bk_d)
            bv_sb = cst.tile([1, 256], F32)
            nc.sync.dma_start(bv_sb[:, :], bv_d)

            wqs = cst.tile([128, 2, 8], F32)
            nc.vector.reduce_sum(out=wqs[:, :, :],
                                 in_=wq_sb[:, :, :].rearrange("p c (h z) -> p c h z", z=DK),
                                 axis=mybir.AxisListType.X)
            wks = cst.tile([128, 2, 8], F32)
            nc.vector.reduce_sum(out=wks[:, :, :],
                                 in_=wk_sb[:, :, :].rearrange("p c (h z) -> p c h z", z=DK),
                                 axis=mybir.AxisListType.X)
            bqs_row = cst.tile([1, 8], F32)
            nc.vector.reduce_sum(out=bqs_row[:, :],
                                 in_=bq_sb[:, :].rearrange("o (h z) -> o h z", z=DK),
                                 axis=mybir.AxisListType.X)
            bks_row = cst.tile([1, 8], F32)
            nc.vector.reduce_sum(out=bks_row[:, :],
                                 in_=bk_sb[:, :].rearrange("o (h z) -> o h z", z=DK),
                                 axis=mybir.AxisListType.X)
            bqs_ps = ps_tp.tile([8, 1], F32, tag="tp")
            nc.tensor.matmul(bqs_ps[:, :], bqs_row[:, :], one1[:, :], start=True, stop=True)
            bqs_vert = cst.tile([8, 1], F32)
            nc.vector.tensor_copy(bqs_vert[:, :], bqs_ps[:, :])
            bks_ps = ps_tp.tile([8, 1], F32, tag="tp")
            nc.tensor.matmul(bks_ps[:, :], bks_row[:, :], one1[:, :], start=True, stop=True)
            bks_vert = cst.tile([8, 1], F32)
            nc.vector.tensor_copy(bks_vert[:, :], bks_ps[:, :])

            # this core's batch of v, bf16 (gpsimd cast-DMA; first gpsimd instr)
            v_all = cst.tile([128, 8, 256], BF16)
            nc.gpsimd.dma_start(v_all[:, :, :],
                                v_d.rearrange("(t p) d -> p t d", p=128))

            # ---- single-batch q/k projections ----
            xsT = {}
            for (nat_src, w_sum, bias_v, nm) in (
                (q_d, wqs, bqs_vert, "q"),
                (k_d, wks, bks_vert, "k"),
            ):
                nat = work.tile([128, 8, 256], F32, tag="nat")
                nc.sync.dma_start(nat[:, :, :], nat_src.rearrange("(t p) d -> p t d", p=128))
                tr = work.tile([128, 2, L], F32, tag="tr")
                for t in range(8):
                    for c in range(2):
                        tp = ps_tp.tile([128, 128], F32, tag="tp")
                        nc.tensor.transpose(tp[:, :], nat[:, t, 128 * c:128 * (c + 1)], ident128[:, :])
                        nc.vector.tensor_copy(tr[:, c, 128 * t:128 * (t + 1)], tp[:, :])
                xs = work.tile([8, L], F32, tag=f"{nm}sT")
                for half in range(2):
                    sl = slice(512 * half, 512 * (half + 1))
                    ps_x = ps_out.tile([8, 512], F32, tag="po")
                    nc.tensor.matmul(ps_x[:, :], w_sum[:, 0, :], tr[:, 0, sl], start=True, stop=False)
                    nc.tensor.matmul(ps_x[:, :], w_sum[:, 1, :], tr[:, 1, sl], start=False, stop=True)
                    nc.vector.tensor_scalar(
                        out=xs[:, sl], in0=ps_x[:, :],
                        scalar1=bias_v[:, 0:1], scalar2=None, op0=mybir.AluOpType.add)
                xsT[nm] = xs

            prod = work.tile([8, L], F32, tag="prod")
            nc.vector.tensor_mul(prod[:, :], xsT["q"][:, :], xsT["k"][:, :])
            # corr row: r[0, l] = SCALE * sum_h prod[h, l]
            sones = cst.tile([8, 1], F32)
            nc.vector.memset(sones[:, :], SCALE)
            ps_r = ps_big.tile([1, L], F32, tag="big")
            for half in range(2):
                sl = slice(512 * half, 512 * (half + 1))
                nc.tensor.matmul(ps_r[:, sl], sones[:, :], prod[:, sl], start=True, stop=True)
            r_sb = cst.tile([1, L], F32)
            nc.vector.tensor_copy(r_sb[:, :], ps_r[:, :])

            topv = cst.tile([1, 8], F32)
            nc.vector.max(topv[:, :], r_sb[:, :])
            negm0 = cst.tile([1, 1], F32)
            nc.vector.tensor_scalar_mul(negm0[:, :], topv[:, 0:1], -1.0)
            e_sb = cst.tile([1, K_TOP], F32)
            nc.scalar.activation(e_sb[:, :], topv[:, 0:K_TOP],
                                 mybir.ActivationFunctionType.Exp,
                                 bias=negm0[:, 0:1], scale=1.0)
            z_sb = cst.tile([1, 1], F32)
            nc.vector.reduce_sum(out=z_sb[:, :], in_=e_sb[:, :], axis=mybir.AxisListType.X)
            zinv = cst.tile([1, 1], F32)
            nc.vector.reciprocal(zinv[:, :], z_sb[:, :])
            w_sb = cst.tile([1, K_TOP], F32)
            nc.vector.tensor_scalar_mul(w_sb[:, :], e_sb[:, :], zinv[:, 0:1])

            selu = cst.tile([1, L], F32)
            ohw = cst.tile([1, L], F32)
            for j in range(K_TOP):
                dst = selu if j == 0 else ohw
                nc.vector.tensor_scalar(
                    out=dst[:, :], in0=r_sb[:, :],
                    scalar1=topv[:, j:j + 1], scalar2=w_sb[:, j:j + 1],
                    op0=mybir.AluOpType.is_equal, op1=mybir.AluOpType.mult)
                if j > 0:
                    nc.vector.tensor_add(selu[:, :], selu[:, :], ohw[:, :])

            # selT[p, t] = selu[0, 128 t + p] via K=1 matmuls
            selT = cst.tile([128, 8], F32)
            for t in range(8):
                tps = ps_tp.tile([128, 1], F32, tag="tp")
                nc.tensor.matmul(tps[:, :], selu[0:1, 128 * t:128 * (t + 1)], one1[:, :],
                                 start=True, stop=True)
                nc.vector.tensor_copy(selT[:, t:t + 1], tps[:, :])

            # acc[p, e] = sum_t v[128 t + p, e] * sel[128 t + p]   (DVE)
            acc = cst.tile([128, 256], F32)
            nc.vector.tensor_scalar(out=acc[:, :], in0=v_all[:, 0, :],
                                    scalar1=selT[:, 0:1], scalar2=None,
                                    op0=mybir.AluOpType.mult)
            tmp_ac = cst.tile([128, 256], F32)
            for t in range(1, 8):
                nc.vector.tensor_scalar(out=tmp_ac[:, :], in0=v_all[:, t, :],
                                        scalar1=selT[:, t:t + 1], scalar2=None,
                                        op0=mybir.AluOpType.mult)
                nc.vector.tensor_add(acc[:, :], acc[:, :], tmp_ac[:, :])

            # vbarT[e] = sum_p acc[p, e]  -> [128, 2] (e chunks)
            vbarT = cst.tile([128, 2], F32)
            for m in range(2):
                pv = ps_tp.tile([128, 1], F32, tag="tp")
                nc.tensor.matmul(pv[:, :], acc[:, 128 * m:128 * (m + 1)], ones128[:, :],
                                 start=True, stop=True)
                nc.vector.tensor_copy(vbarT[:, m:m + 1], pv[:, :])

            # agg[d'] = sum_e Wv[e, d'] vbarT[e] + bv[d']  -> [128, 2] (d' chunks)
            agg_sb = cst.tile([128, 2], F32)
            for m in range(2):
                pa = ps_tp.tile([128, 1], F32, tag="tp")
                nc.tensor.matmul(pa[:, :], wv_sb[:, 0, 128 * m:128 * (m + 1)],
                                 vbarT[:, 0:1], start=True, stop=False)
                nc.tensor.matmul(pa[:, :], wv_sb[:, 1, 128 * m:128 * (m + 1)],
                                 vbarT[:, 1:2], start=False, stop=False)
                nc.tensor.matmul(pa[:, :], bv_sb[0:1, 128 * m:128 * (m + 1)],
                                 one1[:, :], start=False, stop=True)
                nc.vector.tensor_copy(agg_sb[:, m:m + 1], pa[:, :])

            # AllGather agg -> [8, 256]
            agg_in = dr.tile([1, D], F32)
            nc.sync.dma_start(
                agg_in[:, :].rearrange("o (m e) -> (o e) m", e=128), agg_sb[:, :])
            agg_out = dr.tile([B, D], F32)
            nc.gpsimd.collective_compute(
                "AllGather", mybir.AluOpType.bypass,
                replica_groups=[list(range(N_CORES))],
                ins=[agg_in[:, :].opt()], outs=[agg_out[:, :].opt()])
            aggf = cst.tile([8, 256], F32)
            nc.sync.dma_start(aggf[:, :], agg_out[:, :])
            aggt_bf = cst.tile([128, 16], BF16)
            for m in range(2):
                pt = ps_tp.tile([128, 8], F32, tag="tp")
                nc.tensor.transpose(pt[:, :], aggf[0:8, 128 * m:128 * (m + 1)], ident8[:, :])
                nc.vector.tensor_copy(aggt_bf[:, 8 * m:8 * (m + 1)], pt[:, :])

            # ---- big output projection; wp loads f32 on sync, DVE casts to bf16 ----
            for nt in range(N_TILES):
                ncol = slice(TILE_N * nt, TILE_N * (nt + 1))
                wp0f = wpp.tile([128, TILE_N], F32, tag="wp0f", bufs=3)
                nc.sync.dma_start(wp0f[:, :], wp_d[0:128, ncol])
                wp1f = wpp.tile([128, TILE_N], F32, tag="wp1f", bufs=3)
                nc.sync.dma_start(wp1f[:, :], wp_d[128:256, ncol])
                wp0 = wpp.tile([128, TILE_N], BF16, tag="wp0")
                nc.vector.tensor_copy(wp0[:, :], wp0f[:, :])
                wp1 = wpp.tile([128, TILE_N], BF16, tag="wp1")
                nc.vector.tensor_copy(wp1[:, :], wp1f[:, :])
                bp_t = wpp.tile([1, TILE_N], BF16, tag="bp", bufs=2)
                nc.gpsimd.dma_start(bp_t[:, :], bp_d[0:1, ncol])
                bp_rep = wpp.tile([8, TILE_N], BF16, tag="bprep", bufs=2)
                nc.gpsimd.partition_broadcast(bp_rep[:, :], bp_t[:, :])
                o_sb = outp.tile([8, TILE_N], F32)
                for s in range(SUBS):
                    ssl = slice(512 * s, 512 * (s + 1))
                    ps = ps_out.tile([8, 512], F32, tag="po")
                    nc.tensor.matmul(ps[:, :], aggt_bf[:, 0:8], wp0[:, ssl], start=True, stop=False)
                    nc.tensor.matmul(ps[:, :], aggt_bf[:, 8:16], wp1[:, ssl], start=False, stop=True)
                    if s % 2 == 0:
                        nc.scalar.copy(o_sb[:, ssl], ps[:, :])
                    else:
                        nc.vector.tensor_copy(o_sb[:, ssl], ps[:, :])
                nc.vector.tensor_add(o_sb[:, :], o_sb[:, :], bp_rep[:, :])
                nc.sync.dma_start(out_d[:, ncol], o_sb[:, :])

    nc.finalize()
    return nc


def _get_nc():
    if "nc" not in _CACHE:
        _CACHE["nc"] = _build_nc_dp() if MODE == "dp" else _build_nc()
    return _CACHE["nc"]


def kernel(queries, keys, values, Wq, bq, Wk, bk, Wv, bv, Wp, bp):
    queries = np.ascontiguousarray(np.asarray(queries, np.float32).reshape(B * L, D))
    keys = np.ascontiguousarray(np.asarray(keys, np.float32).reshape(B * L, D))
    values = np.ascontiguousarray(np.asarray(values, np.float32).reshape(B * L, D))
    Wq = np.ascontiguousarray(np.asarray(Wq, np.float32))
    Wk = np.ascontiguousarray(np.asarray(Wk, np.float32))
    Wv = np.ascontiguousarray(np.asarray(Wv, np.float32))
    bq = np.asarray(bq, np.float32).reshape(1, D)
    bk = np.asarray(bk, np.float32).reshape(1, D)
    bv = np.asarray(bv, np.float32).reshape(1, D)
    Wp = np.asarray(Wp, np.float32)
    bp = np.asarray(bp, np.float32)

    nc = _get_nc()
    qT = np.ascontiguousarray(
        queries.reshape(B, L, D).transpose(0, 2, 1).reshape(B * D, L))
    kT = np.ascontiguousarray(
        keys.reshape(B, L, D).transpose(0, 2, 1).reshape(B * D, L))
    blk3_const = np.zeros((8, 8, 8), np.float32)
    for b in range(B):
        blk3_const[:, b, b] = SCALE
    in_maps = []
    for i in range(N_CORES):
        cols = slice(NSH * i, NSH * (i + 1))
        m = {
            "wq": Wq, "wk": Wk, "wv": Wv,
            "bq": bq, "bk": bk, "bv": bv,
            "wp": np.ascontiguousarray(Wp[:, cols]),
        }
        if MODE == "dp":
            m["bp"] = np.ascontiguousarray(bp[cols]).reshape(1, NSH)
        else:
            m["bp8"] = np.ascontiguousarray(
                np.broadcast_to(bp[cols], (B, NSH)))
        if MODE == "dp":
            rows = slice(L * i, L * (i + 1))
            m.update({"q": queries[rows], "k": keys[rows], "v": values[rows]})
        else:
            m.update({"qt": qT, "kt": kT, "v": values, "blk3": blk3_const})
        in_maps.append(m)
    res = run_bass_kernel_spmd(nc, in_maps, core_ids=list(range(N_CORES)), trace=TRACE)
    global LAST_RESULT
    LAST_RESULT = res
    out = np.concatenate([res.results[i]["out"] for i in range(N_CORES)], axis=1)
    return out.reshape(B, L, D)


# revision 19
# speedup vs baseline: 1.5574x; 1.2989x over previous
"""AutoCorrelation layer kernel for 8 Trainium2 NeuronCores.

Math note: the reference's rfft/irfft pair over the zero-padded head dim
computes a circular cross-correlation; its mean over all lags collapses
analytically to (sum_d q_proj) * (sum_d k_proj) per head.  So
corr_mean[b,l] = (1/(H*L)) * sum_h (q[b,l] @ WqS + bqS)_h * (k[b,l] @ WkS + bkS)_h
with WqS = Wq.reshape(D,H,DK).sum(-1).  Everything downstream (top-6,
softmax, gather, output projection) follows the reference directly.

Distribution: all 8 cores redundantly compute the cheap preprocessing
(full q/k/v, ~24MB) and each core computes its own column shard of the
huge (256, 262144) output projection (column-parallel, no collectives).
"""
import sys
import types

sys.path.insert(0, "/opt/trn_rl_repo")

import numpy as np
import concourse.bass as bass
import concourse.mybir as mybir
import concourse.tile as tile
from concourse import bacc
from concourse.bass_utils import run_bass_kernel_spmd
from concourse.masks import make_identity

F32 = mybir.dt.float32
BF16 = mybir.dt.bfloat16

N_CORES = 8
B, L, D, H, DK = 8, 1024, 256, 8, 32
K_TOP = 6
NSH = (L * D) // N_CORES          # 32768 output cols per core
TILE_N = 2048
N_TILES = NSH // TILE_N           # 16
SUBS = TILE_N // 512              # 4
SCALE = 1.0 / (H * L)

WP_BUFS = 5
MODE = "rep"  # "dp" = batch-parallel preproc + AllGather; "rep" = redundant preproc
DEBUG_OUTS = False
TRACE = False          # test harness sets this for profiled runs
LAST_RESULT = None     # stashed BassKernelResults from the last kernel() call

_CACHE = {}


def _build_nc():
    nc = bacc.Bacc("TRN2", target_bir_lowering=False, debug=False, num_devices=N_CORES)

    qt_d = nc.dram_tensor("qt", [B * D, L], F32, kind="ExternalInput").ap()
    kt_d = nc.dram_tensor("kt", [B * D, L], F32, kind="ExternalInput").ap()
    v_d = nc.dram_tensor("v", [B * L, D], F32, kind="ExternalInput").ap()
    wq_d = nc.dram_tensor("wq", [D, D], F32, kind="ExternalInput").ap()
    wk_d = nc.dram_tensor("wk", [D, D], F32, kind="ExternalInput").ap()
    wv_d = nc.dram_tensor("wv", [D, D], F32, kind="ExternalInput").ap()
    bq_d = nc.dram_tensor("bq", [1, D], F32, kind="ExternalInput").ap()
    bk_d = nc.dram_tensor("bk", [1, D], F32, kind="ExternalInput").ap()
    bv_d = nc.dram_tensor("bv", [1, D], F32, kind="ExternalInput").ap()
    wp_d = nc.dram_tensor("wp", [D, NSH], F32, kind="ExternalInput").ap()
    bp8_d = nc.dram_tensor("bp8", [B, NSH], F32, kind="ExternalInput").ap()
    out_d = nc.dram_tensor("out", [B, NSH], F32, kind="ExternalOutput").ap()
    if DEBUG_OUTS:
        dbg_r = nc.dram_tensor("dbg_r", [B, L], F32, kind="ExternalOutput").ap()
        dbg_aggt = nc.dram_tensor("dbg_aggt", [128, 16], F32, kind="ExternalOutput").ap()

    with tile.TileContext(nc) as tc:
        with (
            tc.tile_pool(name="cst", bufs=1) as cst,
            tc.tile_pool(name="work", bufs=2) as work,
            tc.tile_pool(name="wpp", bufs=WP_BUFS) as wpp,
            tc.tile_pool(name="outp", bufs=3) as outp,
            tc.tile_pool(name="ps_tp", bufs=2, space="PSUM") as ps_tp,
            tc.tile_pool(name="ps_big", bufs=1, space="PSUM") as ps_big,
            tc.tile_pool(name="ps_out", bufs=4, space="PSUM") as ps_out,
        ):
            # ---------------- constants / weights ----------------
            ident128 = cst.tile([128, 128], F32)
            make_identity(nc, ident128[:, :])
            ident8 = cst.tile([8, 8], F32)
            make_identity(nc, ident8[:, :])
            one1 = cst.tile([1, 1], F32)
            nc.vector.memset(one1[:, :], 1.0)
            ones8f = cst.tile([1, 8], F32)
            nc.vector.memset(ones8f[:, :], 1.0)
            ones8b = cst.tile([1, 8], BF16)
            nc.vector.memset(ones8b[:, :], 1.0)
            # blk3[h, b, m] = SCALE * (m == b): per-batch column selector for the
            # corr reduction over heads (host-provided structural constant)
            blk3_d = nc.dram_tensor("blk3", [8, 8, 8], F32, kind="ExternalInput").ap()
            blk3 = cst.tile([8, 8, 8], F32)
            nc.sync.dma_start(blk3[:, :, :], blk3_d)

            wq_sb = cst.tile([128, 2, 256], F32)
            nc.sync.dma_start(wq_sb[:, :, :], wq_d.rearrange("(c p) d -> p c d", p=128))
            wk_sb = cst.tile([128, 2, 256], F32)
            nc.sync.dma_start(wk_sb[:, :, :], wk_d.rearrange("(c p) d -> p c d", p=128))
            wv_sb = cst.tile([128, 2, 256], F32)
            nc.sync.dma_start(wv_sb[:, :, :], wv_d.rearrange("(c p) d -> p c d", p=128))
            bq_sb = cst.tile([1, 256], F32)
            nc.sync.dma_start(bq_sb[:, :], bq_d)
            bk_sb = cst.tile([1, 256], F32)
            nc.sync.dma_start(bk_sb[:, :], bk_d)
            bv_sb = cst.tile([1, 256], F32)
            nc.sync.dma_start(bv_sb[:, :], bv_d)

            # head-sums of projection weights: WqS[d, h] = sum_z Wq[d, h*32+z]
            wqs = cst.tile([128, 2, 8], F32)
            nc.vector.reduce_sum(out=wqs[:, :, :],
                                 in_=wq_sb[:, :, :].rearrange("p c (h z) -> p c h z", z=DK),
                                 axis=mybir.AxisListType.X)
            wks = cst.tile([128, 2, 8], F32)
            nc.vector.reduce_sum(out=wks[:, :, :],
                                 in_=wk_sb[:, :, :].rearrange("p c (h z) -> p c h z", z=DK),
                                 axis=mybir.AxisListType.X)
            bqs_row = cst.tile([1, 8], F32)
            nc.vector.reduce_sum(out=bqs_row[:, :],
                                 in_=bq_sb[:, :].rearrange("o (h z) -> o h z", z=DK),
                                 axis=mybir.AxisListType.X)
            bks_row = cst.tile([1, 8], F32)
            nc.vector.reduce_sum(out=bks_row[:, :],
                                 in_=bk_sb[:, :].rearrange("o (h z) -> o h z", z=DK),
                                 axis=mybir.AxisListType.X)
            # [1,8] -> [8,1] via K=1 matmul against [1,1] ones
            bqs_ps = ps_tp.tile([8, 1], F32, tag="tp")
            nc.tensor.matmul(bqs_ps[:, :], bqs_row[:, :], one1[:, :], start=True, stop=True)
            bqs_vert = cst.tile([8, 1], F32)
            nc.vector.tensor_copy(bqs_vert[:, :], bqs_ps[:, :])
            bks_ps = ps_tp.tile([8, 1], F32, tag="tp")
            nc.tensor.matmul(bks_ps[:, :], bks_row[:, :], one1[:, :], start=True, stop=True)
            bks_vert = cst.tile([8, 1], F32)
            nc.vector.tensor_copy(bks_vert[:, :], bks_ps[:, :])

            # v (bf16, cast in DMA) for the weighted gather
            v_all = cst.tile([128, B, 8, 256], BF16)
            nc.gpsimd.dma_start(v_all[:, :, :, :],
                                v_d.rearrange("(b t p) d -> p b t d", p=128, t=8))

            # ---------------- per-batch q/k projections ----------------
            ps_r = ps_big.tile([8, L], F32, tag="big")
            for b in range(B):
                xsT = {}
                for (t_src, w_sum, bias_v, nm) in (
                    (qt_d, wqs, bqs_vert, "q"),
                    (kt_d, wks, bks_vert, "k"),
                ):
                    # host provides x^T per batch: rows [256 b : 256 (b+1)] are [d, l]
                    tr = work.tile([128, 2, L], F32, tag="tr", bufs=4)
                    nc.sync.dma_start(
                        tr[:, :, :],
                        t_src[D * b:D * (b + 1), :].rearrange("(c p) l -> p c l", p=128))
                    # project: xsT[h, l] = sum_d WS[d, h] * xT[d, l]
                    xs = work.tile([8, L], F32, tag=f"{nm}sT")
                    for half in range(2):
                        sl = slice(512 * half, 512 * (half + 1))
                        ps_x = ps_out.tile([8, 512], F32, tag="po")
                        nc.tensor.matmul(ps_x[:, :], w_sum[:, 0, :], tr[:, 0, sl], start=True, stop=False)
                        nc.tensor.matmul(ps_x[:, :], w_sum[:, 1, :], tr[:, 1, sl], start=False, stop=True)
                        # psum->sbuf with per-head bias add
                        nc.vector.tensor_scalar(
                            out=xs[:, sl], in0=ps_x[:, :],
                            scalar1=bias_v[:, 0:1], scalar2=None, op0=mybir.AluOpType.add)
                    xsT[nm] = xs
                # prod_b[h, l] then accumulate into corr rows via blk3 selector
                prod = work.tile([8, L], F32, tag="prod")
                nc.vector.tensor_mul(prod[:, :], xsT["q"][:, :], xsT["k"][:, :])
                for half in range(2):
                    sl = slice(512 * half, 512 * (half + 1))
                    nc.tensor.matmul(ps_r[:, sl], blk3[:, b, :], prod[:, sl],
                                     start=(b == 0), stop=(b == B - 1))

            # ---------------- corr, top-6, softmax, select ----------------
            r_sb = cst.tile([8, L], F32)
            nc.vector.tensor_copy(r_sb[:, :], ps_r[:, :])
            if DEBUG_OUTS:
                nc.sync.dma_start(dbg_r, r_sb[:, :])

            topv = cst.tile([8, 8], F32)
            nc.vector.max(topv[:, :], r_sb[:, :])
            negm0 = cst.tile([8, 1], F32)
            nc.vector.tensor_scalar_mul(negm0[:, :], topv[:, 0:1], -1.0)
            e_sb = cst.tile([8, K_TOP], F32)
            nc.scalar.activation(e_sb[:, :], topv[:, 0:K_TOP],
                                 mybir.ActivationFunctionType.Exp,
                                 bias=negm0[:, 0:1], scale=1.0)
            z_sb = cst.tile([8, 1], F32)
            nc.vector.reduce_sum(out=z_sb[:, :], in_=e_sb[:, :], axis=mybir.AxisListType.X)
            zinv = cst.tile([8, 1], F32)
            nc.vector.reciprocal(zinv[:, :], z_sb[:, :])
            w_sb = cst.tile([8, K_TOP], F32)
            nc.vector.tensor_scalar_mul(w_sb[:, :], e_sb[:, :], zinv[:, 0:1])

            # selu[b, l] = sum_j w_j * (r[b, l] == topv[b, j])
            selu = cst.tile([8, L], F32)
            ohw = cst.tile([8, L], F32)
            for j in range(K_TOP):
                dst = selu if j == 0 else ohw
                nc.vector.tensor_scalar(
                    out=dst[:, :], in0=r_sb[:, :],
                    scalar1=topv[:, j:j + 1], scalar2=w_sb[:, j:j + 1],
                    op0=mybir.AluOpType.is_equal, op1=mybir.AluOpType.mult)
                if j > 0:
                    nc.vector.tensor_add(selu[:, :], selu[:, :], ohw[:, :])

            # transpose sel to [l_local, t] layout (bf16), t = l // 128
            selT = cst.tile([128, 64], BF16)
            for t in range(8):
                tp8 = ps_tp.tile([128, 8], F32, tag="tp")
                nc.tensor.transpose(tp8[:, :], selu[0:8, 128 * t:128 * (t + 1)], ident8[:, :])
                nc.vector.tensor_copy(selT[:, 8 * t:8 * (t + 1)], tp8[:, :])

            # vbarT[e, b] = sum_l v[b, l, e] * sel[b, l]
            # DVE: acc[p, e] = sum_t v[b, 128 t + p, e] * sel[b, 128 t + p]
            vbarT = cst.tile([128, 16], F32)
            for b in range(B):
                for e in range(2):
                    pv = ps_tp.tile([128, 1], F32, tag="tp")
                    for t in range(8):
                        nc.tensor.matmul(pv[:, :],
                                         v_all[:, b, t, 128 * e:128 * (e + 1)],
                                         selT[:, 8 * t + b:8 * t + b + 1],
                                         start=(t == 0), stop=(t == 7))
                    nc.vector.tensor_copy(vbarT[:, 8 * e + b:8 * e + b + 1], pv[:, :])

            # aggT[d', b] = sum_e Wv[e, d'] * vbarT[e, b] + bv[d']   (bf16 out)
            aggt_bf = cst.tile([128, 16], BF16)
            for m in range(2):
                ps_a = ps_tp.tile([128, 8], F32, tag="tp")
                nc.tensor.matmul(ps_a[:, :], wv_sb[:, 0, 128 * m:128 * (m + 1)],
                                 vbarT[:, 0:8], start=True, stop=False)
                nc.tensor.matmul(ps_a[:, :], wv_sb[:, 1, 128 * m:128 * (m + 1)],
                                 vbarT[:, 8:16], start=False, stop=False)
                nc.tensor.matmul(ps_a[:, :], bv_sb[0:1, 128 * m:128 * (m + 1)],
                                 ones8f[:, :], start=False, stop=True)
                nc.vector.tensor_copy(aggt_bf[:, 8 * m:8 * (m + 1)], ps_a[:, :])
            if DEBUG_OUTS:
                aggt_f = cst.tile([128, 16], F32)
                nc.vector.tensor_copy(aggt_f[:, :], aggt_bf[:, :])
                nc.sync.dma_start(dbg_aggt, aggt_f[:, :])

            # ---------------- big output projection (column shard) ----------------
            for nt in range(N_TILES):
                ncol = slice(TILE_N * nt, TILE_N * (nt + 1))
                wp0 = wpp.tile([128, TILE_N], BF16, tag="wp0")
                nc.gpsimd.dma_start(wp0[:, :], wp_d[0:128, ncol])
                wp1 = wpp.tile([128, TILE_N], BF16, tag="wp1")
                nc.gpsimd.dma_start(wp1[:, :], wp_d[128:256, ncol])
                bp_rep = wpp.tile([8, TILE_N], F32, tag="bprep", bufs=2)
                nc.sync.dma_start(bp_rep[:, :], bp8_d[:, ncol])
                o_sb = outp.tile([8, TILE_N], F32)
                for s in range(SUBS):
                    ssl = slice(512 * s, 512 * (s + 1))
                    ps = ps_out.tile([8, 512], F32, tag="po")
                    nc.tensor.matmul(ps[:, :], aggt_bf[:, 0:8], wp0[:, ssl], start=True, stop=False)
                    nc.tensor.matmul(ps[:, :], aggt_bf[:, 8:16], wp1[:, ssl], start=False, stop=True)
                    if s % 2 == 0:
                        nc.scalar.copy(o_sb[:, ssl], ps[:, :])
                    else:
                        nc.vector.tensor_copy(o_sb[:, ssl], ps[:, :])
                nc.vector.tensor_add(o_sb[:, :], o_sb[:, :], bp_rep[:, :])
                nc.sync.dma_start(out_d[:, ncol], o_sb[:, :])

    nc.finalize()
    return nc


def _build_nc_dp():
    """Batch-parallel variant: core i preprocesses batch i only, then an
    AllGather of the tiny agg vector feeds the column-sharded projection."""
    nc = bacc.Bacc("TRN2", target_bir_lowering=False, debug=False, num_devices=N_CORES)

    q_d = nc.dram_tensor("q", [L, D], F32, kind="ExternalInput").ap()
    k_d = nc.dram_tensor("k", [L, D], F32, kind="ExternalInput").ap()
    v_d = nc.dram_tensor("v", [L, D], F32, kind="ExternalInput").ap()
    wq_d = nc.dram_tensor("wq", [D, D], F32, kind="ExternalInput").ap()
    wk_d = nc.dram_tensor("wk", [D, D], F32, kind="ExternalInput").ap()
    wv_d = nc.dram_tensor("wv", [D, D], F32, kind="ExternalInput").ap()
    bq_d = nc.dram_tensor("bq", [1, D], F32, kind="ExternalInput").ap()
    bk_d = nc.dram_tensor("bk", [1, D], F32, kind="ExternalInput").ap()
    bv_d = nc.dram_tensor("bv", [1, D], F32, kind="ExternalInput").ap()
    wp_d = nc.dram_tensor("wp", [D, NSH], F32, kind="ExternalInput").ap()
    bp_d = nc.dram_tensor("bp", [1, NSH], F32, kind="ExternalInput").ap()
    out_d = nc.dram_tensor("out", [B, NSH], F32, kind="ExternalOutput").ap()

    with tile.TileContext(nc) as tc:
        with (
            tc.tile_pool(name="cst", bufs=1) as cst,
            tc.tile_pool(name="work", bufs=2) as work,
            tc.tile_pool(name="wpp", bufs=WP_BUFS) as wpp,
            tc.tile_pool(name="outp", bufs=3) as outp,
            tc.tile_pool(name="dr", bufs=1, space="DRAM") as dr,
            tc.tile_pool(name="ps_tp", bufs=2, space="PSUM") as ps_tp,
            tc.tile_pool(name="ps_big", bufs=1, space="PSUM") as ps_big,
            tc.tile_pool(name="ps_out", bufs=2, space="PSUM") as ps_out,
        ):
            ident128 = cst.tile([128, 128], F32)
            make_identity(nc, ident128[:, :])
            ident8 = cst.tile([8, 8], F32)
            make_identity(nc, ident8[:, :])
            one1 = cst.tile([1, 1], F32)
            nc.vector.memset(one1[:, :], 1.0)
            ones8b = cst.tile([1, 8], BF16)
            nc.vector.memset(ones8b[:, :], 1.0)
            ones128 = cst.tile([128, 1], F32)
            nc.vector.memset(ones128[:, :], 1.0)

            wq_sb = cst.tile([128, 2, 256], F32)
            nc.sync.dma_start(wq_sb[:, :, :], wq_d.rearrange("(c p) d -> p c d", p=128))
            wk_sb = cst.tile([128, 2, 256], F32)
            nc.sync.dma_start(wk_sb[:, :, :], wk_d.rearrange("(c p) d -> p c d", p=128))
            wv_sb = cst.tile([128, 2, 256], F32)
            nc.sync.dma_start(wv_sb[:, :, :], wv_d.rearrange("(c p) d -> p c d", p=128))
            bq_sb = cst.tile([1, 256], F32)
            nc.sync.dma_start(bq_sb[:, :], bq_d)
            bk_sb = cst.tile([1, 256], F32)
            nc.sync.dma_start(bk_sb[:, :], bk_d)
            bv_sb = cst.tile([1, 256], F32)
            nc.sync.dma_start(bv_sb[:, :], bv_d)

            wqs = cst.tile([128, 2, 8], F32)
            nc.vector.reduce_sum(out=wqs[:, :, :],
                                 in_=wq_sb[:, :, :].rearrange("p c (h z) -> p c h z", z=DK),
                                 axis=mybir.AxisListType.X)
            wks = cst.tile([128, 2, 8], F32)
            nc.vector.reduce_sum(out=wks[:, :, :],
                                 in_=wk_sb[:, :, :].rearrange("p c (h z) -> p c h z", z=DK),
                                 axis=mybir.AxisListType.X)
            bqs_row = cst.tile([1, 8], F32)
            nc.vector.reduce_sum(out=bqs_row[:, :],
                                 in_=bq_sb[:, :].rearrange("o (h z) -> o h z", z=DK),
                                 axis=mybir.AxisListType.X)
            bks_row = cst.tile([1, 8], F32)
            nc.vector.reduce_sum(out=bks_row[:, :],
                                 in_=bk_sb[:, :].rearrange("o (h z) -> o h z", z=DK),
                                 axis=mybir.AxisListType.X)
            bqs_ps = ps_tp.tile([8, 1], F32, tag="tp")
            nc.tensor.matmul(bqs_ps[:, :], bqs_row[:, :], one1[:, :], start=True, stop=True)
            bqs_vert = cst.tile([8, 1], F32)
            nc.vector.tensor_copy(bqs_vert[:, :], bqs_ps[:, :])
            bks_ps = ps_tp.tile([8, 1], F32, tag="tp")
            nc.tensor.matmul(bks_ps[:, :], bks_row[:, :], one1[:, :], start=True, stop=True)
            bks_vert = cst.tile([8, 1], F32)
            nc.vector.tensor_copy(bks_vert[:, :], bks_ps[:, :])

            # this core's batch of v, bf16 (gpsimd cast-DMA; first gpsimd instr)
            v_all = cst.tile([128, 8, 256], BF16)
            nc.gpsimd.dma_start(v_all[:, :, :],
                                v_d.rearrange("(t p) d -> p t d", p=128))

            # ---- single-batch q/k projections ----
            xsT = {}
            for (nat_src, w_sum, bias_v, nm) in (
                (q_d, wqs, bqs_vert, "q"),
                (k_d, wks, bks_vert, "k"),
            ):
                nat = work.tile([128, 8, 256], F32, tag="nat")
                nc.sync.dma_start(nat[:, :, :], nat_src.rearrange("(t p) d -> p t d", p=128))
                tr = work.tile([128, 2, L], F32, tag="tr", bufs=4)
                for t in range(8):
                    for c in range(2):
                        tp = ps_tp.tile([128, 128], F32, tag="tp")
                        nc.tensor.transpose(tp[:, :], nat[:, t, 128 * c:128 * (c + 1)], ident128[:, :])
                        nc.vector.tensor_copy(tr[:, c, 128 * t:128 * (t + 1)], tp[:, :])
                xs = work.tile([8, L], F32, tag=f"{nm}sT")
                for half in range(2):
                    sl = slice(512 * half, 512 * (half + 1))
                    ps_x = ps_out.tile([8, 512], F32, tag="po")
                    nc.tensor.matmul(ps_x[:, :], w_sum[:, 0, :], tr[:, 0, sl], start=True, stop=False)
                    nc.tensor.matmul(ps_x[:, :], w_sum[:, 1, :], tr[:, 1, sl], start=False, stop=True)
                    nc.vector.tensor_scalar(
                        out=xs[:, sl], in0=ps_x[:, :],
                        scalar1=bias_v[:, 0:1], scalar2=None, op0=mybir.AluOpType.add)
                xsT[nm] = xs

            prod = work.tile([8, L], F32, tag="prod")
            nc.vector.tensor_mul(prod[:, :], xsT["q"][:, :], xsT["k"][:, :])
            # corr row: r[0, l] = SCALE * sum_h prod[h, l]
            sones = cst.tile([8, 1], F32)
            nc.vector.memset(sones[:, :], SCALE)
            ps_r = ps_big.tile([1, L], F32, tag="big")
            for half in range(2):
                sl = slice(512 * half, 512 * (half + 1))
                nc.tensor.matmul(ps_r[:, sl], sones[:, :], prod[:, sl], start=True, stop=True)
            r_sb = cst.tile([1, L], F32)
            nc.vector.tensor_copy(r_sb[:, :], ps_r[:, :])

            topv = cst.tile([1, 8], F32)
            nc.vector.max(topv[:, :], r_sb[:, :])
            negm0 = cst.tile([1, 1], F32)
            nc.vector.tensor_scalar_mul(negm0[:, :], topv[:, 0:1], -1.0)
            e_sb = cst.tile([1, K_TOP], F32)
            nc.scalar.activation(e_sb[:, :], topv[:, 0:K_TOP],
                                 mybir.ActivationFunctionType.Exp,
                                 bias=negm0[:, 0:1], scale=1.0)
            z_sb = cst.tile([1, 1], F32)
            nc.vector.reduce_sum(out=z_sb[:, :], in_=e_sb[:, :], axis=mybir.AxisListType.X)
            zinv = cst.tile([1, 1], F32)
            nc.vector.reciprocal(zinv[:, :], z_sb[:, :])
            w_sb = cst.tile([1, K_TOP], F32)
            nc.vector.tensor_scalar_mul(w_sb[:, :], e_sb[:, :], zinv[:, 0:1])

            selu = cst.tile([1, L], F32)
            ohw = cst.tile([1, L], F32)
            for j in range(K_TOP):
                dst = selu if j == 0 else ohw
                nc.vector.tensor_scalar(
                    out=dst[:, :], in0=r_sb[:, :],
                    scalar1=topv[:, j:j + 1], scalar2=w_sb[:, j:j + 1],
                    op0=mybir.AluOpType.is_equal, op1=mybir.AluOpType.mult)
                if j > 0:
                    nc.vector.tensor_add(selu[:, :], selu[:, :], ohw[:, :])

            # selT[p, t] = selu[0, 128 t + p] via K=1 matmuls
            selT = cst.tile([128, 8], F32)
            for t in range(8):
                tps = ps_tp.tile([128, 1], F32, tag="tp")
                nc.tensor.matmul(tps[:, :], selu[0:1, 128 * t:128 * (t + 1)], one1[:, :],
                                 start=True, stop=True)
                nc.vector.tensor_copy(selT[:, t:t + 1], tps[:, :])

            # acc[p, e] = sum_t v[128 t + p, e] * sel[128 t + p]   (DVE)
            acc = cst.tile([128, 256], F32)
            nc.vector.tensor_scalar(out=acc[:, :], in0=v_all[:, 0, :],
                                    scalar1=selT[:, 0:1], scalar2=None,
                                    op0=mybir.AluOpType.mult)
            tmp_ac = cst.tile([128, 256], F32)
            for t in range(1, 8):
                nc.vector.tensor_scalar(out=tmp_ac[:, :], in0=v_all[:, t, :],
                                        scalar1=selT[:, t:t + 1], scalar2=None,
                                        op0=mybir.AluOpType.mult)
                nc.vector.tensor_add(acc[:, :], acc[:, :], tmp_ac[:, :])

            # vbarT[e] = sum_p acc[p, e]  -> [128, 2] (e chunks)
            vbarT = cst.tile([128, 2], F32)
            for m in range(2):
                pv = ps_tp.tile([128, 1], F32, tag="tp")
                nc.tensor.matmul(pv[:, :], acc[:, 128 * m:128 * (m + 1)], ones128[:, :],
                                 start=True, stop=True)
                nc.vector.tensor_copy(vbarT[:, m:m + 1], pv[:, :])

            # agg[d'] = sum_e Wv[e, d'] vbarT[e] + bv[d']  -> [128, 2] (d' chunks)
            agg_sb = cst.tile([128, 2], F32)
            for m in range(2):
                pa = ps_tp.tile([128, 1], F32, tag="tp")
                nc.tensor.matmul(pa[:, :], wv_sb[:, 0, 128 * m:128 * (m + 1)],
                                 vbarT[:, 0:1], start=True, stop=False)
                nc.tensor.matmul(pa[:, :], wv_sb[:, 1, 128 * m:128 * (m + 1)],
                                 vbarT[:, 1:2], start=False, stop=False)
                nc.tensor.matmul(pa[:, :], bv_sb[0:1, 128 * m:128 * (m + 1)],
                                 one1[:, :], start=False, stop=True)
                nc.vector.tensor_copy(agg_sb[:, m:m + 1], pa[:, :])

            # AllGather agg -> [8, 256]
            agg_in = dr.tile([1, D], F32)
            nc.sync.dma_start(
                agg_in[:, :].rearrange("o (m e) -> (o e) m", e=128), agg_sb[:, :])
            agg_out = dr.tile([B, D], F32)
            nc.gpsimd.collective_compute(
                "AllGather", mybir.AluOpType.bypass,
                replica_groups=[list(range(N_CORES))],
                ins=[agg_in[:, :].opt()], outs=[agg_out[:, :].opt()])
            aggf = cst.tile([8, 256], F32)
            nc.sync.dma_start(aggf[:, :], agg_out[:, :])
            aggt_bf = cst.tile([128, 16], BF16)
            for m in range(2):
                pt = ps_tp.tile([128, 8], F32, tag="tp")
                nc.tensor.transpose(pt[:, :], aggf[0:8, 128 * m:128 * (m + 1)], ident8[:, :])
                nc.vector.tensor_copy(aggt_bf[:, 8 * m:8 * (m + 1)], pt[:, :])

            # ---- big output projection; wp loads f32 on sync, DVE casts to bf16 ----
            for nt in range(N_TILES):
                ncol = slice(TILE_N * nt, TILE_N * (nt + 1))
                wp0f = wpp.tile([128, TILE_N], F32, tag="wp0f", bufs=3)
                nc.sync.dma_start(wp0f[:, :], wp_d[0:128, ncol])
                wp1f = wpp.tile([128, TILE_N], F32, tag="wp1f", bufs=3)
                nc.sync.dma_start(wp1f[:, :], wp_d[128:256, ncol])
                wp0 = wpp.tile([128, TILE_N], BF16, tag="wp0")
                nc.vector.tensor_copy(wp0[:, :], wp0f[:, :])
                wp1 = wpp.tile([128, TILE_N], BF16, tag="wp1")
                nc.vector.tensor_copy(wp1[:, :], wp1f[:, :])
                bp_t = wpp.tile([1, TILE_N], BF16, tag="bp", bufs=2)
                nc.gpsimd.dma_start(bp_t[:, :], bp_d[0:1, ncol])
                bp_rep = wpp.tile([8, TILE_N], BF16, tag="bprep", bufs=2)
                nc.gpsimd.partition_broadcast(bp_rep[:, :], bp_t[:, :])
                o_sb = outp.tile([8, TILE_N], F32)
                for s in range(SUBS):
                    ssl = slice(512 * s, 512 * (s + 1))
                    ps = ps_out.tile([8, 512], F32, tag="po")
                    nc.tensor.matmul(ps[:, :], aggt_bf[:, 0:8], wp0[:, ssl], start=True, stop=False)
                    nc.tensor.matmul(ps[:, :], aggt_bf[:, 8:16], wp1[:, ssl], start=False, stop=True)
                    if s % 2 == 0:
                        nc.scalar.copy(o_sb[:, ssl], ps[:, :])
                    else:
                        nc.vector.tensor_copy(o_sb[:, ssl], ps[:, :])
                nc.vector.tensor_add(o_sb[:, :], o_sb[:, :], bp_rep[:, :])
                nc.sync.dma_start(out_d[:, ncol], o_sb[:, :])

    nc.finalize()
    return nc


def _get_nc():
    if "nc" not in _CACHE:
        _CACHE["nc"] = _build_nc_dp() if MODE == "dp" else _build_nc()
    return _CACHE["nc"]


def kernel(queries, keys, values, Wq, bq, Wk, bk, Wv, bv, Wp, bp):
    queries = np.ascontiguousarray(np.asarray(queries, np.float32).reshape(B * L, D))
    keys = np.ascontiguousarray(np.asarray(keys, np.float32).reshape(B * L, D))
    values = np.ascontiguousarray(np.asarray(values, np.float32).reshape(B * L, D))
    Wq = np.ascontiguousarray(np.asarray(Wq, np.float32))
    Wk = np.ascontiguousarray(np.asarray(Wk, np.float32))
    Wv = np.ascontiguousarray(np.asarray(Wv, np.float32))
    bq = np.asarray(bq, np.float32).reshape(1, D)
    bk = np.asarray(bk, np.float32).reshape(1, D)
    bv = np.asarray(bv, np.float32).reshape(1, D)
    Wp = np.asarray(Wp, np.float32)
    bp = np.asarray(bp, np.float32)

    nc = _get_nc()
    qT = np.ascontiguousarray(
        queries.reshape(B, L, D).transpose(0, 2, 1).reshape(B * D, L))
    kT = np.ascontiguousarray(
        keys.reshape(B, L, D).transpose(0, 2, 1).reshape(B * D, L))
    blk3_const = np.zeros((8, 8, 8), np.float32)
    for b in range(B):
        blk3_const[:, b, b] = SCALE
    in_maps = []
    for i in range(N_CORES):
        cols = slice(NSH * i, NSH * (i + 1))
        m = {
            "wq": Wq, "wk": Wk, "wv": Wv,
            "bq": bq, "bk": bk, "bv": bv,
            "wp": np.ascontiguousarray(Wp[:, cols]),
        }
        if MODE == "dp":
            m["bp"] = np.ascontiguousarray(bp[cols]).reshape(1, NSH)
        else:
            m["bp8"] = np.ascontiguousarray(
                np.broadcast_to(bp[cols], (B, NSH)))
        if MODE == "dp":
            rows = slice(L * i, L * (i + 1))
            m.update({"q": queries[rows], "k": keys[rows], "v": values[rows]})
        else:
            m.update({"qt": qT, "kt": kT, "v": values, "blk3": blk3_const})
        in_maps.append(m)
    res = run_bass_kernel_spmd(nc, in_maps, core_ids=list(range(N_CORES)), trace=TRACE)
    global LAST_RESULT
    LAST_RESULT = res
    out = np.concatenate([res.results[i]["out"] for i in range(N_CORES)], axis=1)
    return out.reshape(B, L, D)
